# revision 46
# baseline (speedup 1.0000x reference)
"""Trainium2 Bass kernel for nn_BDHEncoder (hierarchical vision transformer).

Key ideas:
- Linear attention: (qk qk^T) v == qk (qk^T v)  (no softmax in reference),
  so attention is O(N*hd^2) instead of O(N^2*hd).
- Data-parallel over the 4 images (cores 0-3; 4-7 duplicates).
- Residual kept token-major [128, nT, D] (cheap per-token LN via bn_stats +
  Newton-rsqrt, no ACT table); activations feature-major [128, nD, T] for
  weight-stationary matmuls; PE transposes bridge, packed 8-16 per PSUM bank
  with one grouped evacuation each.
- RoPE via 2 host tables; the even/odd de-interleave + half-swap live in the
  qk weight rows (second "swapped" qk matmul), 1/sqrt(hd) folded into tables.
- LN gamma/beta folded into enc/gate weights; BN folded into convs; biases
  fused into PSUM-evac ops (ACT activation bias / DVE scalar_tensor_tensor).
- Only ACT table function used is Sigmoid -> single table load.
- bf16 everywhere, f32 PSUM/stats.

Self-contained: hardcodes all shapes from the problem spec.
"""
import sys

if '/opt/trn_rl_repo' not in sys.path:
    sys.path.insert(0, '/opt/trn_rl_repo')

import numpy as np
import ml_dtypes

import concourse.bass as bass
import concourse.mybir as mybir
import concourse.tile as tile
from concourse import bacc

BF16 = ml_dtypes.bfloat16
DT = mybir.dt
OP = mybir.AluOpType
AF = mybir.ActivationFunctionType

DEPTHS = [2, 2, 4, 2]
DIMS = [64, 128, 256, 512]
HEADS = 4
EPS = 1e-5
HWS = [56, 28, 14, 7]
TS = [h * h for h in HWS]
B = 4
# stage 1 runs "stacked": tokens folded in half onto the partition axis,
# so it looks like a D=128, T=1568 stage with block-diagonal weights.
D_EFF = [128, 128, 256, 512]
T_EFF = [1568, 784, 196, 49]
HALVES = [2, 1, 1, 1]
NCHUNK = 512
MAGIC = 0x5F3759DF
NEWTON_ITERS = 2
LAYERS = ['enc', 'gate', 'qk', 'qks', 'v', 'out']


def _tiles(T):
    return [min(128, T - j * 128) for j in range((T + 127) // 128)]


def _chunks(T, c=NCHUNK):
    return [(t0, min(c, T - t0)) for t0 in range(0, T, c)]


# ============================================================ host-side prep

def f32(a):
    return np.asarray(a, dtype=np.float32)


def _rope_perm(D, hd):
    p = []
    for h in range(D // hd):
        p += [h * hd + i for i in range(0, hd, 2)]
        p += [h * hd + i for i in range(1, hd, 2)]
    return np.array(p)


def _swap_perm(D, hd):
    p = []
    for h in range(D // hd):
        p += list(range(h * hd + hd // 2, h * hd + hd))
        p += list(range(h * hd, h * hd + hd // 2))
    return np.array(p)


def rope_tables(T, D, hd):
    inv_freq = 1.0 / (10000.0 ** (np.arange(0, D, 2, dtype=np.float64) / D))
    freqs = np.arange(T, dtype=np.float64)[:, None] * inv_freq[None, :]
    base = np.cos(freqs[:, :hd]).astype(np.float32)
    cos_t, sin_t = np.cos(base), np.sin(base)
    A, Bb = cos_t[:, 0::2], sin_t[:, 0::2]
    C, Dd = sin_t[:, 1::2], cos_t[:, 1::2]
    s = float(hd) ** -0.25
    ta = np.tile(np.concatenate([A, Dd], 1).T * s, (D // hd, 1))
    tb = np.tile(np.concatenate([-Bb, C], 1).T * s, (D // hd, 1))
    return ta.astype(np.float32), tb.astype(np.float32)


def fold_block(p, D):
    g, b = f32(p['ln_g']), f32(p['ln_b'])
    hd = D // HEADS
    pr, sw = _rope_perm(D, hd), _swap_perm(D, hd)
    enc_w, gate_w = f32(p['enc_w']), f32(p['gate_w'])
    qk_w, v_w, out_w = f32(p['qk_w']), f32(p['v_w']), f32(p['out_w'])
    return dict(
        enc=((enc_w * g[None, :]).T, f32(p['enc_b']) + enc_w @ b),
        gate=((gate_w * g[None, :]).T, f32(p['gate_b']) + gate_w @ b),
        qk=(qk_w[pr].T, f32(p['qk_b'])[pr]),
        qks=(qk_w[pr][sw].T, f32(p['qk_b'])[pr][sw]),
        v=(v_w.T, f32(p['v_b'])),
        out=(out_w.T, f32(p['out_b'])))


def wblock_pack(fw, D):
    """Pack block weights -> bf16 [pdim, cols], biases -> f32 [128, bcols]."""
    if D == 64:
        w = np.concatenate([
            np.concatenate([fw['enc'][0], fw['gate'][0]], 1),
            np.concatenate([fw['qk'][0], fw['qks'][0]], 1),
            fw['v'][0], fw['out'][0]], axis=1)      # [64, 384]
        bias = np.zeros((128, 5), np.float32)
        bias[:, 0] = np.concatenate([fw['enc'][1], fw['gate'][1]])
        bias[:, 1] = np.concatenate([fw['qk'][1], fw['qks'][1]])
        bias[:64, 2] = fw['v'][1]
        bias[:64, 3] = fw['out'][1]
        bias[:64, 4] = fw['qks'][1]
        return w.astype(BF16), bias
    n = D // 128
    mats, bias = [], np.zeros((128, n, 6), np.float32)
    for li, l in enumerate(LAYERS):
        W, bv = fw[l]
        mats.append(W.reshape(n, 128, n, 128).transpose(1, 0, 2, 3).reshape(128, -1))
        bias[:, :, li] = bv.reshape(n, 128).T
    return np.concatenate(mats, 1).astype(BF16), bias.reshape(128, -1)


def fold_conv(d):
    w, b = f32(d['conv_w']), f32(d['conv_b'])
    inv = f32(d['bn_g']) / np.sqrt(f32(d['bn_v']) + EPS)
    return (w * inv[:, None, None, None],
            b * inv + f32(d['bn_b']) - f32(d['bn_m']) * inv)


def conv_pack(w2, Cin, Cout):
    nKi, nMo = max(1, Cin // 128), Cout // 128
    pi = min(Cin, 128)
    out = np.zeros((pi, nKi, 4, nMo, 128), np.float32)
    for dy in range(2):
        for dx in range(2):
            wt = w2[:, :, dy, dx].T
            out[:, :, dy * 2 + dx, :, :] = \
                wt.reshape(nKi, pi, nMo, 128).transpose(1, 0, 2, 3)
    return out.reshape(pi, -1).astype(BF16)


def im2col_patch(x_img):
    c = np.asarray(x_img, np.float32).reshape(3, 56, 4, 56, 4)
    p = c.transpose(0, 2, 4, 1, 3).reshape(48, 3136)
    return np.concatenate([p[:, :1568], p[:, 1568:]], axis=0)  # [96, 1568]


def blockdiag_mask(D, hd):
    g = min(D, 128)
    m = np.zeros((g, g), np.float32)
    for h0 in range(0, g, hd):
        m[h0:h0 + hd, h0:h0 + hd] = 1.0
    return m.astype(BF16)


TS_MODE = (3,)   # stages using token-stationary matmuls


def wblock_pack_ts(fw, D, ta, tb, T):
    """ts-mode: weights as moving rhs [128, L, nKi, D]; biases as:
    encb [128, nK] f32 (per-partition, applied at transpose evac);
    bbc [128, 3, D] bf16 broadcast rows (gate, v, out);
    bsum [128, nT, D] bf16 = qk_b*ta + qks_b*tb (token-major)."""
    n = D // 128
    mats = np.zeros((128, n, 6, D), np.float32)
    for li, l in enumerate(LAYERS):
        mats[:, :, li, :] = fw[l][0].reshape(n, 128, D).transpose(1, 0, 2)
    encb = fw['enc'][1].reshape(n, 128).T.astype(np.float32).copy()
    bbc = np.zeros((128, 3, D), np.float32)
    bbc[:, 0, :] = fw['gate'][1][None, :]
    bbc[:, 1, :] = fw['v'][1][None, :]
    bbc[:, 2, :] = fw['out'][1][None, :]
    bs = fw['qk'][1][None, :] * ta.T + fw['qks'][1][None, :] * tb.T  # [T, D]
    nT = (T + 127) // 128
    bs = np.pad(bs, ((0, nT * 128 - T), (0, 0))).reshape(nT, 128, D)
    bsum = bs.transpose(1, 0, 2)
    return (mats.reshape(128, -1).astype(BF16), encb,
            bbc.reshape(128, -1).astype(BF16),
            bsum.reshape(128, -1).astype(BF16).copy())


def _stack2(a):
    """block-diag stack of a [n, m] -> [2n, 2m]"""
    n, m = a.shape
    o = np.zeros((2 * n, 2 * m), a.dtype)
    o[:n, :m] = a
    o[n:, m:] = a
    return o


def prep_consts(params):
    c = {}
    pw = f32(params['patch_w']).reshape(64, 48).T    # [48, 64]
    c['patchw'] = _stack2(pw).astype(BF16).copy()    # [96, 128]
    pb = f32(params['patch_b'])
    c['patchb'] = np.concatenate([pb, pb]).reshape(128, 1).copy()
    for s, D in enumerate(DIMS):
        hd = D // HEADS
        De, Te = D_EFF[s], T_EFF[s]
        nD = De // 128
        pdim = 128
        ta, tb = rope_tables(TS[s], D, hd)
        if s == 0:
            ta = np.concatenate([ta[:, :Te], ta[:, Te:]], axis=0)  # [128, 1568]
            tb = np.concatenate([tb[:, :Te], tb[:, Te:]], axis=0)
        c[f'ta{s}'] = ta.reshape(nD, pdim, Te).transpose(1, 0, 2).astype(BF16).copy()
        c[f'tb{s}'] = tb.reshape(nD, pdim, Te).transpose(1, 0, 2).astype(BF16).copy()
        if s < 3:
            c[f'mask{s}'] = blockdiag_mask(D_EFF[s] if s == 0 else D, hd)
        for bi in range(DEPTHS[s]):
            fw = fold_block(params['stages'][s][bi], D)
            if s == 0:
                fw = {k: (_stack2(w), np.concatenate([b, b]))
                      for k, (w, b) in fw.items()}
            if s in TS_MODE:
                w, encb, bbc, bsum = wblock_pack_ts(fw, D, ta, tb, TS[s])
                c[f'wts{s}_{bi}'] = w
                c[f'encb{s}_{bi}'] = encb
                c[f'bbc{s}_{bi}'] = bbc
                c[f'bsum{s}_{bi}'] = bsum
            else:
                w, bias = wblock_pack(fw, D_EFF[s])
                c[f'w{s}_{bi}'] = w
                c[f'bias{s}_{bi}'] = bias
        if s in TS_MODE:
            nT = (TS[s] + 127) // 128
            pad = nT * 128 - TS[s]
            tap = np.pad(ta.T, ((0, pad), (0, 0))).reshape(nT, 128, D)
            tbp = np.pad(tb.T, ((0, pad), (0, 0))).reshape(nT, 128, D)
            c[f'tatm{s}'] = tap.transpose(1, 0, 2).astype(BF16).copy()
            c[f'tbtm{s}'] = tbp.transpose(1, 0, 2).astype(BF16).copy()
        if s < 3:
            w2, b2 = fold_conv(params['down'][s])
            cp = conv_pack(w2, DIMS[s], DIMS[s + 1])
            if s == 0:
                cp = np.concatenate([cp, cp], axis=0)  # dup rows for half B
            c[f'conv{s}'] = cp
            c[f'convb{s}'] = b2.reshape(-1, 128).T.astype(np.float32).copy()
    c['ident'] = np.eye(128, dtype=BF16)
    i2 = np.zeros((128, 64), BF16)
    i2[:64] = np.eye(64, dtype=BF16)
    i2[64:] = np.eye(64, dtype=BF16)
    c['ident2'] = i2
    c['ones'] = np.ones((1, 128), dtype=BF16)
    c['magic'] = np.full((128, 32), MAGIC, np.int32)
    return c


# ========================================================== device program

class KB:
    """Kernel builder context."""

    def __init__(self, nc, tc):
        self.nc = nc
        self.tc = tc
        self._rr = 0

    def evac_copy(self, out, in_):
        """Round-robin DVE/ACT for PSUM-source copies (GPSIMD can't read PSUM);
        biased 2:1 toward ACT, which carries less elementwise load."""
        self._rr += 1
        if self._rr % 3 == 0:
            return self.nc.vector.tensor_copy(out=out, in_=in_)
        return self.nc.scalar.copy(out=out, in_=in_)


def build_program():
    nc = bacc.Bacc("TRN2", target_bir_lowering=False, debug=True)
    ins = {}

    def din(name, shape, dt=DT.bfloat16):
        ins[name] = nc.dram_tensor(name, shape, dt, kind="ExternalInput")

    din('xpatch', [96, T_EFF[0]])
    din('patchw', [96, 128])
    din('patchb', [128, 1], DT.float32)
    din('ident', [128, 128])
    din('ident2', [128, 64])
    din('ones', [1, 128])
    din('magic', [128, 32], DT.int32)
    for s, D in enumerate(DIMS):
        De, Te = D_EFF[s], T_EFF[s]
        nD = De // 128
        pdim = min(D, 128)
        din(f'ta{s}', [128, nD, Te])
        din(f'tb{s}', [128, nD, Te])
        if s < 3:
            mp = 128 if s == 0 else pdim
            din(f'mask{s}', [mp, mp])
        wcols = nD * nD * 128 * 6
        for bi in range(DEPTHS[s]):
            if s in TS_MODE:
                nTd = (TS[s] + 127) // 128
                din(f'wts{s}_{bi}', [128, 6 * nD * D])
                din(f'encb{s}_{bi}', [128, nD], DT.float32)
                din(f'bbc{s}_{bi}', [128, 3 * D])
                din(f'bsum{s}_{bi}', [128, nTd * D])
            else:
                din(f'w{s}_{bi}', [128, wcols])
                din(f'bias{s}_{bi}', [128, nD * 6], DT.float32)
        if s in TS_MODE:
            nT = (TS[s] + 127) // 128
            din(f'tatm{s}', [128, nT, D])
            din(f'tbtm{s}', [128, nT, D])
        if s < 3:
            nMo = DIMS[s + 1] // 128
            din(f'conv{s}', [128 if s == 0 else pdim, max(1, D // 128) * 4 * nMo * 128])
            din(f'convb{s}', [128, nMo], DT.float32)

    outs = {s: nc.dram_tensor(f'feat{s}', [T_EFF[s], D_EFF[s]], DT.bfloat16,
                              kind="ExternalOutput")
            for s in range(4)}
    import os
    if os.environ.get('KDBG'):
        for i in range(8):
            outs[f'dbg{i}'] = nc.dram_tensor(f'dbg{i}', [128, 4096], DT.float32,
                                             kind="ExternalOutput")

    with tile.TileContext(nc) as tc:
        _emit(nc, tc, ins, outs)
    nc.compile()
    return nc


def _emit(nc, tc, ins, outs):
    from contextlib import ExitStack
    with ExitStack() as ctx:
        kb = KB(nc, tc)
        kb.consts = ctx.enter_context(tc.tile_pool(name="consts", bufs=1))
        kb.wpool = ctx.enter_context(tc.tile_pool(name="wpool", bufs=2))
        kb.work = ctx.enter_context(tc.tile_pool(name="work", bufs=1))
        kb.resid = ctx.enter_context(tc.tile_pool(name="resid", bufs=2))
        kb.small = ctx.enter_context(tc.tile_pool(name="small", bufs=2))
        kb.ps_mm = ctx.enter_context(tc.tile_pool(name="ps_mm", bufs=4, space="PSUM"))
        kb.ps_tr = ctx.enter_context(tc.tile_pool(name="ps_tr", bufs=2, space="PSUM"))
        kb.ps_s = ctx.enter_context(tc.tile_pool(name="ps_s", bufs=2, space="PSUM"))

        # inputs needed first: patch conv operands (chunked)
        xp = kb.work.tile([96, T_EFF[0]], DT.bfloat16, tag="pk_qv")
        for qi, (t0c, tnc) in enumerate(_chunks(T_EFF[0])):
            nc.sync.dma_start(out=xp[:, t0c:t0c + tnc],
                              in_=ins['xpatch'][:, t0c:t0c + tnc])
        pw = kb.consts.tile([96, 128], DT.bfloat16, tag="patchw")
        nc.sync.dma_start(out=pw, in_=ins['patchw'][:, :])
        pb = kb.consts.tile([128, 1], DT.float32, tag="patchb")
        nc.sync.dma_start(out=pb, in_=ins['patchb'][:, :])
        kb.ident = kb.consts.tile([128, 128], DT.bfloat16)
        nc.sync.dma_start(out=kb.ident, in_=ins['ident'][:, :])
        kb.ident2 = kb.consts.tile([128, 64], DT.bfloat16)
        nc.sync.dma_start(out=kb.ident2, in_=ins['ident2'][:, :])
        kb.ones = kb.consts.tile([1, 128], DT.bfloat16)
        nc.sync.dma_start(out=kb.ones, in_=ins['ones'][:, :])
        kb._dmaq = 0

        def pdma(out, in_):
            nc.sync.dma_start(out=out, in_=in_)
        
        kb.magic = kb.consts.tile([128, 32], DT.int32)
        nc.sync.dma_start(out=kb.magic, in_=ins['magic'][:, :])

        # prefetch every weight/bias/table/mask/conv at kernel start
        kb.wt, kb.bt, kb.ropes, kb.masks, kb.convs = {}, {}, {}, {}, {}
        for s, D in enumerate(DIMS):
            De, Te = D_EFF[s], T_EFF[s]
            nD = De // 128
            pdim = min(D, 128)
            nT = (TS[s] + 127) // 128
            for bi in range(DEPTHS[s]):
                if s in TS_MODE:
                    w = kb.consts.tile([128, nD, 6, D], DT.bfloat16,
                                       tag=f"w{s}_{bi}")
                    pdma(out=w, in_=ins[f'wts{s}_{bi}'][:, :].rearrange(
                        "p (k l d) -> p k l d", l=6, k=nD))
                    eb = kb.consts.tile([128, nD], DT.float32, tag=f"encb{s}_{bi}")
                    pdma(out=eb, in_=ins[f'encb{s}_{bi}'][:, :])
                    bbc = kb.consts.tile([128, 3, D], DT.bfloat16, tag=f"bbc{s}_{bi}")
                    pdma(out=bbc, in_=ins[f'bbc{s}_{bi}'][:, :]
                                      .rearrange("p (l d) -> p l d", l=3))
                    bsum = kb.consts.tile([128, nT, D], DT.bfloat16,
                                          tag=f"bsum{s}_{bi}")
                    pdma(out=bsum, in_=ins[f'bsum{s}_{bi}'][:, :]
                                      .rearrange("p (j d) -> p j d", j=nT))
                    kb.bt[(s, bi)] = (eb, bbc, bsum)
                    kb.wt[(s, bi)] = w
                    continue
                if True:
                    w = kb.consts.tile(list(ins[f'w{s}_{bi}'].shape), DT.bfloat16,
                                       tag=f"w{s}_{bi}")
                    pdma(out=w, in_=ins[f'w{s}_{bi}'][:, :])
                    b = kb.consts.tile(list(ins[f'bias{s}_{bi}'].shape), DT.float32,
                                       tag=f"bias{s}_{bi}")
                    pdma(out=b, in_=ins[f'bias{s}_{bi}'][:, :])
                    kb.wt[(s, bi)], kb.bt[(s, bi)] = w, b
            if s in TS_MODE:
                ta = kb.consts.tile([128, nT, DIMS[s]], DT.bfloat16, tag=f"ta{s}")
                pdma(out=ta, in_=ins[f'tatm{s}'][:, :, :])
                tb = kb.consts.tile([128, nT, DIMS[s]], DT.bfloat16, tag=f"tb{s}")
                pdma(out=tb, in_=ins[f'tbtm{s}'][:, :, :])
            else:
                ta = kb.consts.tile([128, nD, Te], DT.bfloat16, tag=f"ta{s}")
                pdma(out=ta, in_=ins[f'ta{s}'][:, :, :])
                tb = kb.consts.tile([128, nD, Te], DT.bfloat16, tag=f"tb{s}")
                pdma(out=tb, in_=ins[f'tb{s}'][:, :, :])
            kb.ropes[s] = (ta, tb)
            if s < 3:
                mp = 128 if s == 0 else pdim
                mk = kb.consts.tile([mp, mp], DT.bfloat16, tag=f"mask{s}")
                pdma(out=mk, in_=ins[f'mask{s}'][:, :])
                kb.masks[s] = mk
                Cin, Cout = D, DIMS[s + 1]
                nKi, nMo = max(1, Cin // 128), Cout // 128
                wc = kb.consts.tile([128 if s == 0 else min(Cin, 128),
                                     nKi, 4, nMo, 128],
                                    DT.bfloat16, tag=f"conv{s}")
                pdma(out=wc, in_=ins[f'conv{s}'][:, :].rearrange(
                    "p (a b c d) -> p a b c d", a=nKi, b=4, c=nMo))
                cb = kb.consts.tile([128, nMo], DT.float32, tag=f"convb{s}")
                pdma(out=cb, in_=ins[f'convb{s}'][:, :])
                kb.convs[s] = (wc, cb)

        # ---- patch conv (feature-major, stacked) then to token-major
        T0 = T_EFF[0]
        x1_fm = kb.work.tile([128, 1, T0], DT.bfloat16, tag="t2_fm")
        for t0, tn in _chunks(T0):
            mm = kb.ps_mm.tile([128, NCHUNK], DT.float32, tag="mm_ps")
            nc.tensor.matmul(mm[:128, :tn], lhsT=pw, rhs=xp[:, t0:t0 + tn],
                             start=True, stop=True)
            nc.scalar.activation(out=x1_fm[:, 0, t0:t0 + tn], in_=mm[:128, :tn],
                                 func=AF.Identity, bias=pb, scale=1.0)
        x_tm = _fm_to_tm(kb, x1_fm, T0, 128, tag_dst="x_s0", pool='resid',
                         zero_ragged=True)
        import os
        if os.environ.get('KDBG'):
            nc.gpsimd.dma_start(out=outs['dbg0'][:, :64], in_=x_tm[:, 0, :])
            nc.gpsimd.dma_start(out=outs['dbg1'][:64, :512], in_=x1_fm[:, 0, :512])
            nc.gpsimd.dma_start(out=outs['dbg2'][:48, :512], in_=xp[:, :512])
            kb.dbg_outs = outs

        # ---- stages
        for s, (depth, D) in enumerate(zip(DEPTHS, DIMS)):
            for bi in range(depth):
                if s in TS_MODE:
                    x_tm = _block_ts(kb, ins, s, bi, x_tm)
                else:
                    x_tm = _block(kb, ins, s, bi, x_tm)
            for j, psz in enumerate(_tiles(T_EFF[s])):
                nc.sync.dma_start(out=outs[s][j * 128:j * 128 + psz, :],
                                  in_=x_tm[:psz, j, :])
            if s < 3:
                x_tm = _downsample(kb, ins, s, x_tm)


def _fm_to_tm(kb, fm, T, D, tag_dst, pool='work', add_to=None,
              zero_ragged=False):
    """Feature-major [pdim, nD, T] -> token-major [128, nT, D] via packed PE
    transposes. If add_to is given, the grouped evac is a TT add with it
    (residual fusion) routed DVE; else a grouped copy (DVE/ACT)."""
    nc = kb.nc
    pdim = min(D, 128)
    nD = max(1, D // 128)
    sizes = _tiles(T)
    nT = len(sizes)
    nfull = sum(1 for p in sizes if p == 128)
    dst = getattr(kb, pool).tile([128, nT, D], DT.bfloat16, tag=tag_dst)
    npack = max(1, 1024 // pdim)

    for g in range(nD):
        gsl = slice(g * 128, g * 128 + pdim)
        for j0 in range(0, nfull, npack):
            jn = min(npack, nfull - j0)
            pt = kb.ps_tr.tile([128, 1024], DT.bfloat16, tag="tr_ps")
            for k in range(jn):
                nc.tensor.transpose(
                    pt[:, k * pdim:(k + 1) * pdim],
                    fm[:, g, (j0 + k) * 128:(j0 + k + 1) * 128],
                    kb.ident[:pdim, :pdim])
            src = pt[:, :jn * pdim].rearrange("p (j d) -> p j d", j=jn)
            dsl = dst[:, j0:j0 + jn, gsl]
            if add_to is not None:
                nc.vector.tensor_tensor(out=dsl, in0=src,
                                        in1=add_to[:, j0:j0 + jn, gsl], op=OP.add)
            else:
                kb.evac_copy(dsl, src)
        if nfull < nT:  # one ragged tail tile
            j = nT - 1
            psz = sizes[j]
            if zero_ragged:
                nc.gpsimd.memset(dst[:, j, gsl], 0.0)
            pt = kb.ps_tr.tile([128, 1024], DT.bfloat16, tag="tr_ps")
            nc.tensor.transpose(pt[:psz, :pdim], fm[:, g, j * 128:j * 128 + psz],
                                kb.ident[:pdim, :pdim])
            dsl = dst[:psz, j, gsl]
            if add_to is not None:
                nc.vector.tensor_tensor(out=dsl, in0=pt[:psz, :pdim],
                                        in1=add_to[:psz, j, gsl], op=OP.add)
            else:
                kb.evac_copy(dsl, pt[:psz, :pdim])
    return dst


def _tm_to_fm(kb, tm, T, D, tag_dst, evac_fn=None):
    """Token-major [128, nT, D] -> feature-major [pdim, nD, T]."""
    nc = kb.nc
    pdim = min(D, 128)
    nD = max(1, D // 128)
    sizes = _tiles(T)
    nT = len(sizes)
    fm = kb.work.tile([pdim, nD, T], DT.bfloat16, tag=tag_dst)
    npack = 8  # 8 * 128 cols = 1024 bf16 = one bank
    for g in range(nD):
        for j0 in range(0, nT, npack):
            jn = min(npack, nT - j0)
            pt = kb.ps_tr.tile([128, 1024], DT.bfloat16, tag="tr_ps")
            cols = 0
            for k in range(jn):
                j = j0 + k
                psz = sizes[j]
                nc.tensor.transpose(
                    pt[:pdim, cols:cols + psz],
                    tm[:psz, j, g * 128:g * 128 + pdim],
                    kb.ident[:psz, :psz])
                cols += psz
            if evac_fn is not None:
                evac_fn(fm[:, g, j0 * 128:j0 * 128 + cols], pt[:pdim, :cols], g)
            else:
                kb.evac_copy(fm[:, g, j0 * 128:j0 * 128 + cols], pt[:pdim, :cols])
    return fm


def _ln(kb, x_tm, T, D, halves=1):
    """LayerNorm stats + normalize -> xn_tm bf16. halves=2: stage-1 stacked
    layout, each 128-col row holds two tokens (64 cols each)."""
    nc = kb.nc
    sizes = _tiles(T)
    nTt = len(sizes)
    nT = nTt * halves
    Dh = D // halves
    st = kb.small.tile([128, nT, 6], DT.float32, tag="bnst")
    for j in range(nTt):
        for h in range(halves):
            nc.vector.bn_stats(out=st[:, j * halves + h, :],
                               in_=x_tm[:, j, h * Dh:(h + 1) * Dh])
    mv = kb.small.tile([128, nT, 2], DT.float32, tag="mv")
    for j in range(nT):
        nc.vector.bn_aggr(out=mv[:, j, :], in_=st[:, j, :])
        if j % 2 == 0:
            ht = kb.ps_s.tile([128, 128], DT.bfloat16, tag="s_ps")
            nc.tensor.transpose(ht, kb.ident, kb.ident)
    veps = kb.small.tile([128, nT], DT.float32, tag="veps")
    nc.vector.tensor_scalar(out=veps, in0=mv[:, :, 1], scalar1=EPS,
                            scalar2=None, op0=OP.add)
    h = kb.small.tile([128, nT], DT.int32, tag="hshift")
    nc.vector.tensor_scalar(out=h, in0=veps.bitcast(DT.int32), scalar1=1,
                            scalar2=None, op0=OP.logical_shift_right)
    y = kb.small.tile([128, nT], DT.float32, tag="ynewt")
    nc.vector.tensor_tensor(out=y.bitcast(DT.int32), in0=kb.magic[:, :nT],
                            in1=h, op=OP.subtract)
    r = kb.small.tile([128, nT], DT.float32, tag="rnewt")
    for _ in range(NEWTON_ITERS):
        nc.vector.tensor_tensor(out=r, in0=y, in1=y, op=OP.mult)
        nc.vector.tensor_tensor(out=r, in0=r, in1=veps, op=OP.mult)
        nc.vector.tensor_scalar(out=r, in0=r, scalar1=-0.5, scalar2=1.5,
                                op0=OP.mult, op1=OP.add)
        nc.vector.tensor_tensor(out=y, in0=y, in1=r, op=OP.mult)
    xn_tm = kb.work.tile([128, nTt, D], DT.bfloat16, tag="xn_tm")
    for j, psz in enumerate(sizes):
        for h in range(halves):
            jh = j * halves + h
            nc.vector.tensor_scalar(out=xn_tm[:psz, j, h * Dh:(h + 1) * Dh],
                                    in0=x_tm[:psz, j, h * Dh:(h + 1) * Dh],
                                    scalar1=mv[:psz, jh, 0:1],
                                    scalar2=y[:psz, jh:jh + 1],
                                    op0=OP.subtract, op1=OP.mult)
    return xn_tm


def _block(kb, ins, s, bi, x_tm):
    nc = kb.nc
    D, T = D_EFF[s], T_EFF[s]
    pdim = 128
    nD = max(1, D // 128)
    sizes = _tiles(T)
    nT = len(sizes)
    n = nD

    wt, bt = kb.wt[(s, bi)], kb.bt[(s, bi)]
    ta, tb = kb.ropes[s]
    if s < 3:
        kb.mask = kb.masks[s]

    def wsl(li, ki, mo):
        if D == 64:
            off = {0: 0, 2: 128, 4: 256, 5: 320}
            wid = {0: 128, 2: 128, 4: 64, 5: 64}
            return wt[:, off[li]:off[li] + wid[li]]
        base = li * n * n * 128
        return wt[:, base + (ki * n + mo) * 128: base + (ki * n + mo + 1) * 128]

    def bsl(li, mo, p0=0, p1=128):
        col = {0: 0, 2: 1, 3: 4, 4: 2, 5: 3}[li] if D == 64 else mo * 6 + li
        return bt[p0:p1, col:col + 1]

    def linear(li, in_fm, dst_fn):
        """dst_fn(mm_psum, mo, t0, tn) consumes each output chunk."""
        mrows = {0: 128, 2: 128, 4: 64, 5: 64}[li] if D == 64 else pdim
        for mo in range(n):
            for t0, tn in _chunks(T):
                mm = kb.ps_mm.tile([128, NCHUNK], DT.float32, tag="mm_ps")
                for ki in range(n):
                    nc.tensor.matmul(mm[:mrows, :tn], lhsT=wsl(li, ki, mo),
                                     rhs=in_fm[:, ki, t0:t0 + tn],
                                     start=(ki == 0), stop=(ki == n - 1))
                dst_fn(mm, mo, t0, tn)

    # ---- LN, transpose
    xn_tm = _ln(kb, x_tm, T, D, halves=HALVES[s])
    xn_fm = _tm_to_fm(kb, xn_tm, T, D, tag_dst="xn_fm")

    # ---- enc(relu) / gate(sigmoid)  [packed for D=64]
    xl_fm = kb.work.tile([pdim, nD, T], DT.bfloat16, tag="xl_fm")
    gsig_fm = kb.work.tile([pdim, nD, T], DT.bfloat16, tag="gsig_fm")
    if D == 64:
        def encgate_dst(mm, mo, t0, tn):
            nc.scalar.activation(out=xl_fm[:, 0, t0:t0 + tn], in_=mm[0:64, :tn],
                                 func=AF.Relu, bias=bsl(0, 0, 0, 64), scale=1.0)
            nc.scalar.activation(out=gsig_fm[:, 0, t0:t0 + tn], in_=mm[64:128, :tn],
                                 func=AF.Sigmoid, bias=bsl(0, 0, 64, 128), scale=1.0)
        linear(0, xn_fm, encgate_dst)
    else:
        def enc_dst(mm, mo, t0, tn):
            nc.scalar.activation(out=xl_fm[:, mo, t0:t0 + tn], in_=mm[:pdim, :tn],
                                 func=AF.Relu, bias=bsl(0, mo), scale=1.0)
        linear(0, xn_fm, enc_dst)

        def gate_dst(mm, mo, t0, tn):
            nc.scalar.activation(out=gsig_fm[:, mo, t0:t0 + tn], in_=mm[:pdim, :tn],
                                 func=AF.Sigmoid, bias=bsl(1, mo), scale=1.0)
        linear(1, xn_fm, gate_dst)

    # ---- qk/qks with fused bias+rope-mult; v with bias
    t1_fm = kb.work.tile([pdim, nD, T], DT.bfloat16, tag="t1_fm")
    t2_fm = kb.work.tile([pdim, nD, T], DT.bfloat16, tag="t2_fm")
    if D == 64:
        def qkqks_dst(mm, mo, t0, tn):
            nc.vector.scalar_tensor_tensor(
                out=t1_fm[:, 0, t0:t0 + tn], in0=mm[0:64, :tn],
                scalar=bsl(2, 0, 0, 64), in1=ta[:, 0, t0:t0 + tn],
                op0=OP.add, op1=OP.mult)
            nc.vector.scalar_tensor_tensor(
                out=t2_fm[:, 0, t0:t0 + tn], in0=mm[64:128, :tn],
                scalar=bsl(3, 0, 0, 64), in1=tb[:, 0, t0:t0 + tn],
                op0=OP.add, op1=OP.mult)
        linear(2, xl_fm, qkqks_dst)
    else:
        def qk_dst(mm, mo, t0, tn):
            nc.vector.scalar_tensor_tensor(
                out=t1_fm[:, mo, t0:t0 + tn], in0=mm[:pdim, :tn],
                scalar=bsl(2, mo), in1=ta[:, mo, t0:t0 + tn],
                op0=OP.add, op1=OP.mult)
        linear(2, xl_fm, qk_dst)

        def qks_dst(mm, mo, t0, tn):
            nc.vector.scalar_tensor_tensor(
                out=t2_fm[:, mo, t0:t0 + tn], in0=mm[:pdim, :tn],
                scalar=bsl(3, mo), in1=tb[:, mo, t0:t0 + tn],
                op0=OP.add, op1=OP.mult)
        linear(3, xl_fm, qks_dst)

    if D == 64:
        # pack qkr (rows 0:64) and v (rows 64:128) into one tile: one
        # transpose covers both for the S stage.
        pk = kb.work.tile([128, 1, T], DT.bfloat16, tag="pk_qv")

        def v_dst(mm, mo, t0, tn):
            nc.scalar.activation(out=pk[64:128, 0, t0:t0 + tn], in_=mm[0:64, :tn],
                                 func=AF.Identity, bias=bsl(4, 0, 0, 64), scale=1.0)
        linear(4, xl_fm, v_dst)
        nc.vector.tensor_tensor(out=pk[0:64, 0, :], in0=t1_fm[:, 0, :],
                                in1=t2_fm[:, 0, :], op=OP.add)
        qv_tm = _fm_to_tm(kb, pk, T, 128, tag_dst="xn_tm")
        qkr_fm = pk
    else:
        v_fm = kb.work.tile([pdim, nD, T], DT.bfloat16, tag="v_fm")

        def v_dst(mm, mo, t0, tn):
            nc.scalar.activation(out=v_fm[:, mo, t0:t0 + tn], in_=mm[:pdim, :tn],
                                 func=AF.Identity, bias=bsl(4, mo, 0, pdim), scale=1.0)
        linear(4, xl_fm, v_dst)

        qkr_fm = kb.work.tile([pdim, nD, T], DT.bfloat16, tag="qkr_fm")
        for g in range(nD):
            eng = nc.vector if g % 2 == 0 else nc.gpsimd
            eng.tensor_tensor(out=qkr_fm[:, g, :], in0=t1_fm[:, g, :],
                              in1=t2_fm[:, g, :], op=OP.add)
        qkr_tm = _fm_to_tm(kb, qkr_fm, T, D, tag_dst="qkr_tm")
        v_tm = _fm_to_tm(kb, v_fm, T, D, tag_dst="v_tm")

    # ---- S per feature group + attn
    attn_fm = kb.work.tile([pdim, nD, T], DT.bfloat16, tag="t1_fm")
    for g in range(nD):
        sp = kb.ps_s.tile([128, 128], DT.float32, tag="s_ps")
        for j, psz in enumerate(sizes):
            if D == 64:
                lhsT = qv_tm[:psz, j, 0:64]
                rhs = qv_tm[:psz, j, 64:128]
            else:
                lhsT = qkr_tm[:psz, j, g * 128:g * 128 + pdim]
                rhs = v_tm[:psz, j, g * 128:g * 128 + pdim]
            nc.tensor.matmul(sp[:pdim, :pdim], lhsT=lhsT, rhs=rhs,
                             start=(j == 0), stop=(j == nT - 1))
        sbd = kb.small.tile([128, 128], DT.bfloat16, tag="sbd")
        if s == 0:
            # stacked: S_full = sum of the two diagonal 64-blocks of sp
            ssb = kb.small.tile([128, 128], DT.bfloat16, tag="ssb")
            nc.vector.tensor_tensor(out=ssb, in0=sp, in1=kb.mask, op=OP.mult)
            rp = kb.ps_s.tile([128, 128], DT.float32, tag="s_ps")
            nc.tensor.matmul(rp[:64, :128], lhsT=kb.ident2, rhs=ssb,
                             start=True, stop=True)
            rs = kb.small.tile([64, 128], DT.bfloat16, tag="rssb")
            kb.evac_copy(rs, rp[:64, :128])
            sfull = kb.small.tile([64, 64], DT.bfloat16, tag="sfull")
            nc.vector.tensor_tensor(out=sfull, in0=rs[:, 0:64], in1=rs[:, 64:128],
                                    op=OP.add)
            nc.vector.memset(sbd, 0.0)
            nc.vector.tensor_copy(out=sbd[0:64, 0:64], in_=sfull)
            nc.vector.tensor_copy(out=sbd[64:128, 64:128], in_=sfull)
        elif s < 3:
            nc.vector.tensor_tensor(out=sbd[:pdim, :pdim], in0=sp[:pdim, :pdim],
                                    in1=kb.mask, op=OP.mult)
        else:
            nc.vector.tensor_copy(out=sbd[:pdim, :pdim], in_=sp[:pdim, :pdim])
        for t0, tn in _chunks(T):
            ap = kb.ps_mm.tile([128, NCHUNK], DT.float32, tag="mm_ps")
            nc.tensor.matmul(ap[:pdim, :tn], lhsT=sbd[:pdim, :pdim],
                             rhs=qkr_fm[0:pdim, g, t0:t0 + tn], start=True, stop=True)
            kb.evac_copy(attn_fm[:, g, t0:t0 + tn], ap[:pdim, :tn])

    # ---- out proj: u = (psum + b) * gsig
    u_fm = kb.work.tile([pdim, nD, T], DT.bfloat16, tag="t2_fm")

    def out_dst(mm, mo, t0, tn):
        nc.vector.scalar_tensor_tensor(
            out=u_fm[:, mo, t0:t0 + tn], in0=mm[:pdim, :tn],
            scalar=bsl(5, mo, 0, pdim), in1=gsig_fm[:, mo, t0:t0 + tn],
            op0=OP.add, op1=OP.mult)
    linear(5, attn_fm, out_dst)

    # ---- residual: x_new = x + u^T (fused into transpose evac)
    x_new = _fm_to_tm(kb, u_fm, T, D, tag_dst=f"x_s{s}", pool='resid',
                      add_to=x_tm, zero_ragged=True)
    import os
    if os.environ.get('KDBG') and s == 0 and bi == 0:
        o = kb.dbg_outs
        nc.gpsimd.dma_start(out=o['dbg3'][:, :64], in_=xn_tm[:, 0, :])
        nc.gpsimd.dma_start(out=o['dbg3'][:, 64:128], in_=xn_tm[:, 1, :])
        nc.gpsimd.dma_start(out=o['dbg3'][:, 128:192], in_=x_tm[:, 1, :])
        nc.gpsimd.dma_start(out=o['dbg3'][:, 192:256], in_=x_tm[:, 2, :])
        nc.gpsimd.dma_start(out=o['dbg4'][:64, :512], in_=xl_fm[:, 0, :512])
        nc.gpsimd.dma_start(out=o['dbg4'][64:128, :512], in_=xn_fm[:, 0, :512])
        nc.gpsimd.dma_start(out=o['dbg5'][:64, :512], in_=qkr_fm[:, 0, :512])
        nc.gpsimd.dma_start(out=o['dbg6'][:64, :512], in_=attn_fm[:, 0, :512])
        nc.gpsimd.dma_start(out=o['dbg7'][:64, :512], in_=u_fm[:, 0, :512])
    return x_new




def _block_ts(kb, ins, s, bi, x_tm):
    """Token-stationary block for small-T stages: activations as lhsT,
    weights as moving rhs, most tensors token-major."""
    nc = kb.nc
    D, T = DIMS[s], TS[s]
    nK = D // 128
    sizes = _tiles(T)
    nT = len(sizes)
    wt = kb.wt[(s, bi)]                  # [128, 6, nK, D]
    encb, bbc, bsum = kb.bt[(s, bi)]     # [128,nK] f32, [128,3,D], [128,nT,D]
    ta, tb = kb.ropes[s]                 # token-major [128, nT, D]

    xn_tm = _ln(kb, x_tm, T, D)
    xn_fm = _tm_to_fm(kb, xn_tm, T, D, tag_dst="xn_fm")

    def linear_grp(grp, in_fm, dst_fn):
        gw = len(grp) * D
        for j, psz in enumerate(sizes):
            mm = kb.ps_mm.tile([128, NCHUNK], DT.float32, tag="mm_ps")
            for ki in range(nK):
                nc.tensor.matmul(mm[:psz, :gw],
                                 lhsT=in_fm[:, ki, j * 128:j * 128 + psz],
                                 rhs=wt[:, ki, grp[0]:grp[0] + len(grp), :],
                                 start=(ki == 0), stop=(ki == nK - 1))
            dst_fn(mm, j, psz)

    pair = (D <= 256)

    # enc(+gate): psum -> xlp_tm (copy) -> transpose with fused Relu+bias evac
    xlp_tm = kb.work.tile([128, nT, D], DT.bfloat16, tag="xl_tm")
    gsig_tm = kb.work.tile([128, nT, D], DT.bfloat16, tag="gsig_tm")

    def gate_cols(mm, j, psz, c0):
        nc.vector.tensor_tensor(out=gsig_tm[:psz, j, :], in0=mm[:psz, c0:c0 + D],
                                in1=bbc[:psz, 0, :], op=OP.add)
        nc.scalar.activation(out=gsig_tm[:psz, j, :], in_=gsig_tm[:psz, j, :],
                             func=AF.Sigmoid)

    if pair:
        def encgate_dst(mm, j, psz):
            kb.evac_copy(xlp_tm[:psz, j, :], mm[:psz, 0:D])
            gate_cols(mm, j, psz, D)
        linear_grp([0, 1], xn_fm, encgate_dst)
    else:
        linear_grp([0], xn_fm, lambda mm, j, psz: kb.evac_copy(
            xlp_tm[:psz, j, :], mm[:psz, :D]))
        linear_grp([1], xn_fm, lambda mm, j, psz: gate_cols(mm, j, psz, 0))
    xl_fm = _tm_to_fm(kb, xlp_tm, T, D, tag_dst="xl_fm",
                      evac_fn=lambda dsl, srcp, g: nc.scalar.activation(
                          out=dsl, in_=srcp, func=AF.Relu,
                          bias=encb[:, g:g + 1], scale=1.0))

    # qk/qks: psum * table -> t1/t2 ; v: psum + bias_bc
    t1_tm = kb.work.tile([128, nT, D], DT.bfloat16, tag="t1_tm")
    t2_tm = kb.work.tile([128, nT, D], DT.bfloat16, tag="t2_tm")
    if pair:
        def qkqks_dst(mm, j, psz):
            nc.vector.tensor_tensor(out=t1_tm[:psz, j, :], in0=mm[:psz, 0:D],
                                    in1=ta[:psz, j, :], op=OP.mult)
            nc.vector.tensor_tensor(out=t2_tm[:psz, j, :], in0=mm[:psz, D:2 * D],
                                    in1=tb[:psz, j, :], op=OP.mult)
        linear_grp([2, 3], xl_fm, qkqks_dst)
    else:
        linear_grp([2], xl_fm, lambda mm, j, psz: nc.vector.tensor_tensor(
            out=t1_tm[:psz, j, :], in0=mm[:psz, :D], in1=ta[:psz, j, :],
            op=OP.mult))
        linear_grp([3], xl_fm, lambda mm, j, psz: nc.vector.tensor_tensor(
            out=t2_tm[:psz, j, :], in0=mm[:psz, :D], in1=tb[:psz, j, :],
            op=OP.mult))
    v_tm = kb.work.tile([128, nT, D], DT.bfloat16, tag="v_tm")
    linear_grp([4], xl_fm, lambda mm, j, psz: nc.vector.tensor_tensor(
        out=v_tm[:psz, j, :], in0=mm[:psz, :D], in1=bbc[:psz, 1, :],
        op=OP.add))
    # qkr = t1 + t2 + bsum
    qkr_tm = kb.work.tile([128, nT, D], DT.bfloat16, tag="qkr_tm")
    for j, psz in enumerate(sizes):
        nc.vector.tensor_tensor(out=qkr_tm[:psz, j, :], in0=t1_tm[:psz, j, :],
                                in1=t2_tm[:psz, j, :], op=OP.add)
        nc.gpsimd.tensor_tensor(out=qkr_tm[:psz, j, :], in0=qkr_tm[:psz, j, :],
                                in1=bsum[:psz, j, :], op=OP.add)

    qkr_fm = _tm_to_fm(kb, qkr_tm, T, D, tag_dst="xl_fm")

    # S + attn (fm result)
    attn_fm = kb.work.tile([128, nK, T], DT.bfloat16, tag="t1_tm")
    for g in range(nK):
        sp = kb.ps_s.tile([128, 128], DT.float32, tag="s_ps")
        for j, psz in enumerate(sizes):
            nc.tensor.matmul(sp, lhsT=qkr_tm[:psz, j, g * 128:g * 128 + 128],
                             rhs=v_tm[:psz, j, g * 128:g * 128 + 128],
                             start=(j == 0), stop=(j == nT - 1))
        sbd = kb.small.tile([128, 128], DT.bfloat16, tag="sbd")
        if s < 3:
            nc.vector.tensor_tensor(out=sbd, in0=sp, in1=kb.masks[s], op=OP.mult)
        else:
            nc.vector.tensor_copy(out=sbd, in_=sp)
        ap = kb.ps_mm.tile([128, NCHUNK], DT.float32, tag="mm_ps")
        nc.tensor.matmul(ap[:128, :T], lhsT=sbd, rhs=qkr_fm[:, g, :],
                         start=True, stop=True)
        kb.evac_copy(attn_fm[:, g, :], ap[:128, :T])

    # out proj + bias + gate + residual
    x_new = kb.resid.tile([128, nT, D], DT.bfloat16, tag=f"x_s{s}")
    u_tm = kb.work.tile([128, nT, D], DT.bfloat16, tag="u_tm")

    def out_dst(mm, j, psz):
        if psz < 128:
            nc.gpsimd.memset(x_new[:, j, :], 0.0)
        nc.vector.tensor_tensor(out=u_tm[:psz, j, :], in0=mm[:psz, :D],
                                in1=bbc[:psz, 2, :], op=OP.add)
        nc.vector.tensor_tensor(out=u_tm[:psz, j, :], in0=u_tm[:psz, j, :],
                                in1=gsig_tm[:psz, j, :], op=OP.mult)
        nc.gpsimd.tensor_tensor(out=x_new[:psz, j, :], in0=u_tm[:psz, j, :],
                                in1=x_tm[:psz, j, :], op=OP.add)
    linear_grp([5], attn_fm, out_dst)
    return x_new


def _downsample(kb, ins, s, x_tm):
    nc = kb.nc
    Cin, Cout = DIMS[s], DIMS[s + 1]
    T, Tn = TS[s], TS[s + 1]
    H, Ho = HWS[s], HWS[s + 1]
    pi = min(Cin, 128)
    nKi, nMo = max(1, Cin // 128), Cout // 128

    wc, cb = kb.convs[s]

    if s == 0:
        x_fm = _tm_to_fm(kb, x_tm, T_EFF[0], 128, tag_dst="gsig_fm")
        y_fm = kb.work.tile([128, nMo, Tn], DT.bfloat16, tag="t2_fm")
        for mo in range(nMo):
            for h in range(2):
                mm = kb.ps_mm.tile([128, NCHUNK], DT.float32, tag="mm_ps")
                imv = x_fm[h * 64:(h + 1) * 64, 0, :].rearrange(
                    "p (ho a wo b) -> p ho a wo b", a=2, b=2, ho=14)
                for k, (dy, dx) in enumerate([(0, 0), (0, 1), (1, 0), (1, 1)]):
                    nc.tensor.matmul(mm[:128, :392],
                                     lhsT=wc[h * 64:(h + 1) * 64, 0, k, mo, :],
                                     rhs=imv[:, :, dy, :, dx],
                                     start=(k == 0), stop=(k == 3))
                nc.scalar.activation(out=y_fm[:, mo, h * 392:(h + 1) * 392],
                                     in_=mm[:128, :392], func=AF.Identity,
                                     bias=cb[:, mo:mo + 1], scale=1.0)
        return _fm_to_tm(kb, y_fm, Tn, Cout, tag_dst="x_s1", pool='resid',
                         zero_ragged=True)

    x_fm = _tm_to_fm(kb, x_tm, T, Cin, tag_dst="gsig_fm")
    y_fm = kb.work.tile([128, nMo, Tn], DT.bfloat16, tag="t2_fm")

    rows = max(1, NCHUNK // Ho)
    for mo in range(nMo):
        for y0 in range(0, Ho, rows):
            yn = min(rows, Ho - y0)
            mm = kb.ps_mm.tile([128, NCHUNK], DT.float32, tag="mm_ps")
            first = True
            for ki in range(nKi):
                imv = x_fm[:, ki, :].rearrange("p (ho a wo b) -> p ho a wo b",
                                               a=2, b=2, ho=H // 2)
                for k, (dy, dx) in enumerate([(0, 0), (0, 1), (1, 0), (1, 1)]):
                    nc.tensor.matmul(mm[:128, :yn * Ho], lhsT=wc[:, ki, k, mo, :],
                                     rhs=imv[:, y0:y0 + yn, dy, :, dx],
                                     start=first, stop=(ki == nKi - 1 and k == 3))
                    first = False
            nc.scalar.activation(out=y_fm[:, mo, y0 * Ho:(y0 + yn) * Ho],
                                 in_=mm[:128, :yn * Ho], func=AF.Identity,
                                 bias=cb[:, mo:mo + 1], scale=1.0)

    return _fm_to_tm(kb, y_fm, Tn, Cout, tag_dst=f"x_s{s + 1}", pool='resid',
                     zero_ragged=True)


# ========================================================== public entry

_CACHE = {}


def _get_program():
    if 'nc' not in _CACHE:
        _CACHE['nc'] = build_program()
    return _CACHE['nc']


def make_in_maps(x, params):
    x = np.asarray(x, dtype=np.float32)
    consts = prep_consts(params)
    in_maps = []
    for core in range(8):
        m = dict(consts)
        m['xpatch'] = im2col_patch(x[core % B]).astype(BF16)
        in_maps.append(m)
    return in_maps


def assemble(results):
    feats = []
    for s, D in enumerate(DIMS):
        H = HWS[s]
        imgs = []
        for b in range(B):
            a = np.asarray(results[b][f'feat{s}']).astype(np.float32)
            if s == 0:
                a = np.concatenate([a[:, :64], a[:, 64:]], axis=0)
            imgs.append(a.T.reshape(D, H, H))
        feats.append(np.stack(imgs))
    return tuple(feats)


def kernel(x, params):
    from concourse.bass_utils import run_bass_kernel_spmd
    nc = _get_program()
    in_maps = make_in_maps(x, params)
    res = run_bass_kernel_spmd(nc, in_maps, list(range(8)))
    return assemble(res.results)


# revision 47
# speedup vs baseline: 1.0020x; 1.0020x over previous
"""Trainium2 Bass kernel for nn_BDHEncoder (hierarchical vision transformer).

Key ideas:
- Linear attention: (qk qk^T) v == qk (qk^T v)  (no softmax in reference),
  so attention is O(N*hd^2) instead of O(N^2*hd).
- Data-parallel over the 4 images (cores 0-3; 4-7 duplicates).
- Residual kept token-major [128, nT, D] (cheap per-token LN via bn_stats +
  Newton-rsqrt, no ACT table); activations feature-major [128, nD, T] for
  weight-stationary matmuls; PE transposes bridge, packed 8-16 per PSUM bank
  with one grouped evacuation each.
- RoPE via 2 host tables; the even/odd de-interleave + half-swap live in the
  qk weight rows (second "swapped" qk matmul), 1/sqrt(hd) folded into tables.
- LN gamma/beta folded into enc/gate weights; BN folded into convs; biases
  fused into PSUM-evac ops (ACT activation bias / DVE scalar_tensor_tensor).
- Only ACT table function used is Sigmoid -> single table load.
- bf16 everywhere, f32 PSUM/stats.

Self-contained: hardcodes all shapes from the problem spec.
"""
import sys

if '/opt/trn_rl_repo' not in sys.path:
    sys.path.insert(0, '/opt/trn_rl_repo')

import numpy as np
import ml_dtypes

import concourse.bass as bass
import concourse.mybir as mybir
import concourse.tile as tile
from concourse import bacc

BF16 = ml_dtypes.bfloat16
DT = mybir.dt
OP = mybir.AluOpType
AF = mybir.ActivationFunctionType

DEPTHS = [2, 2, 4, 2]
DIMS = [64, 128, 256, 512]
HEADS = 4
EPS = 1e-5
HWS = [56, 28, 14, 7]
TS = [h * h for h in HWS]
B = 4
# stage 1 runs "stacked": tokens folded in half onto the partition axis,
# so it looks like a D=128, T=1568 stage with block-diagonal weights.
D_EFF = [128, 128, 256, 512]
T_EFF = [1568, 784, 196, 49]
HALVES = [2, 1, 1, 1]
NCHUNK = 512
MAGIC = 0x5F3759DF
NEWTON_ITERS = 2
LAYERS = ['enc', 'gate', 'qk', 'qks', 'v', 'out']


def _tiles(T):
    return [min(128, T - j * 128) for j in range((T + 127) // 128)]


def _chunks(T, c=NCHUNK):
    return [(t0, min(c, T - t0)) for t0 in range(0, T, c)]


# ============================================================ host-side prep

def f32(a):
    return np.asarray(a, dtype=np.float32)


def _rope_perm(D, hd):
    p = []
    for h in range(D // hd):
        p += [h * hd + i for i in range(0, hd, 2)]
        p += [h * hd + i for i in range(1, hd, 2)]
    return np.array(p)


def _swap_perm(D, hd):
    p = []
    for h in range(D // hd):
        p += list(range(h * hd + hd // 2, h * hd + hd))
        p += list(range(h * hd, h * hd + hd // 2))
    return np.array(p)


def rope_tables(T, D, hd):
    inv_freq = 1.0 / (10000.0 ** (np.arange(0, D, 2, dtype=np.float64) / D))
    freqs = np.arange(T, dtype=np.float64)[:, None] * inv_freq[None, :]
    base = np.cos(freqs[:, :hd]).astype(np.float32)
    cos_t, sin_t = np.cos(base), np.sin(base)
    A, Bb = cos_t[:, 0::2], sin_t[:, 0::2]
    C, Dd = sin_t[:, 1::2], cos_t[:, 1::2]
    s = float(hd) ** -0.25
    ta = np.tile(np.concatenate([A, Dd], 1).T * s, (D // hd, 1))
    tb = np.tile(np.concatenate([-Bb, C], 1).T * s, (D // hd, 1))
    return ta.astype(np.float32), tb.astype(np.float32)


def fold_block(p, D):
    g, b = f32(p['ln_g']), f32(p['ln_b'])
    hd = D // HEADS
    pr, sw = _rope_perm(D, hd), _swap_perm(D, hd)
    enc_w, gate_w = f32(p['enc_w']), f32(p['gate_w'])
    qk_w, v_w, out_w = f32(p['qk_w']), f32(p['v_w']), f32(p['out_w'])
    return dict(
        enc=((enc_w * g[None, :]).T, f32(p['enc_b']) + enc_w @ b),
        gate=((gate_w * g[None, :]).T, f32(p['gate_b']) + gate_w @ b),
        qk=(qk_w[pr].T, f32(p['qk_b'])[pr]),
        qks=(qk_w[pr][sw].T, f32(p['qk_b'])[pr][sw]),
        v=(v_w.T, f32(p['v_b'])),
        out=(out_w.T, f32(p['out_b'])))


def wblock_pack(fw, D):
    """Pack block weights -> bf16 [pdim, cols], biases -> f32 [128, bcols]."""
    if D == 64:
        w = np.concatenate([
            np.concatenate([fw['enc'][0], fw['gate'][0]], 1),
            np.concatenate([fw['qk'][0], fw['qks'][0]], 1),
            fw['v'][0], fw['out'][0]], axis=1)      # [64, 384]
        bias = np.zeros((128, 5), np.float32)
        bias[:, 0] = np.concatenate([fw['enc'][1], fw['gate'][1]])
        bias[:, 1] = np.concatenate([fw['qk'][1], fw['qks'][1]])
        bias[:64, 2] = fw['v'][1]
        bias[:64, 3] = fw['out'][1]
        bias[:64, 4] = fw['qks'][1]
        return w.astype(BF16), bias
    n = D // 128
    mats, bias = [], np.zeros((128, n, 6), np.float32)
    for li, l in enumerate(LAYERS):
        W, bv = fw[l]
        mats.append(W.reshape(n, 128, n, 128).transpose(1, 0, 2, 3).reshape(128, -1))
        bias[:, :, li] = bv.reshape(n, 128).T
    return np.concatenate(mats, 1).astype(BF16), bias.reshape(128, -1)


def fold_conv(d):
    w, b = f32(d['conv_w']), f32(d['conv_b'])
    inv = f32(d['bn_g']) / np.sqrt(f32(d['bn_v']) + EPS)
    return (w * inv[:, None, None, None],
            b * inv + f32(d['bn_b']) - f32(d['bn_m']) * inv)


def conv_pack(w2, Cin, Cout):
    nKi, nMo = max(1, Cin // 128), Cout // 128
    pi = min(Cin, 128)
    out = np.zeros((pi, nKi, 4, nMo, 128), np.float32)
    for dy in range(2):
        for dx in range(2):
            wt = w2[:, :, dy, dx].T
            out[:, :, dy * 2 + dx, :, :] = \
                wt.reshape(nKi, pi, nMo, 128).transpose(1, 0, 2, 3)
    return out.reshape(pi, -1).astype(BF16)


def im2col_patch(x_img):
    c = np.asarray(x_img, np.float32).reshape(3, 56, 4, 56, 4)
    p = c.transpose(0, 2, 4, 1, 3).reshape(48, 3136)
    return np.concatenate([p[:, :1568], p[:, 1568:]], axis=0)  # [96, 1568]


def blockdiag_mask(D, hd):
    g = min(D, 128)
    m = np.zeros((g, g), np.float32)
    for h0 in range(0, g, hd):
        m[h0:h0 + hd, h0:h0 + hd] = 1.0
    return m.astype(BF16)


TS_MODE = (3,)   # stages using token-stationary matmuls


def wblock_pack_ts(fw, D, ta, tb, T):
    """ts-mode: weights as moving rhs [128, L, nKi, D]; biases as:
    encb [128, nK] f32 (per-partition, applied at transpose evac);
    bbc [128, 3, D] bf16 broadcast rows (gate, v, out);
    bsum [128, nT, D] bf16 = qk_b*ta + qks_b*tb (token-major)."""
    n = D // 128
    mats = np.zeros((128, n, 6, D), np.float32)
    for li, l in enumerate(LAYERS):
        mats[:, :, li, :] = fw[l][0].reshape(n, 128, D).transpose(1, 0, 2)
    encb = fw['enc'][1].reshape(n, 128).T.astype(np.float32).copy()
    bbc = np.zeros((128, 3, D), np.float32)
    bbc[:, 0, :] = fw['gate'][1][None, :]
    bbc[:, 1, :] = fw['v'][1][None, :]
    bbc[:, 2, :] = fw['out'][1][None, :]
    bs = fw['qk'][1][None, :] * ta.T + fw['qks'][1][None, :] * tb.T  # [T, D]
    nT = (T + 127) // 128
    bs = np.pad(bs, ((0, nT * 128 - T), (0, 0))).reshape(nT, 128, D)
    bsum = bs.transpose(1, 0, 2)
    return (mats.reshape(128, -1).astype(BF16), encb,
            bbc.reshape(128, -1).astype(BF16),
            bsum.reshape(128, -1).astype(BF16).copy())


def _stack2(a):
    """block-diag stack of a [n, m] -> [2n, 2m]"""
    n, m = a.shape
    o = np.zeros((2 * n, 2 * m), a.dtype)
    o[:n, :m] = a
    o[n:, m:] = a
    return o


def prep_consts(params):
    c = {}
    pw = f32(params['patch_w']).reshape(64, 48).T    # [48, 64]
    c['patchw'] = _stack2(pw).astype(BF16).copy()    # [96, 128]
    pb = f32(params['patch_b'])
    c['patchb'] = np.concatenate([pb, pb]).reshape(128, 1).copy()
    for s, D in enumerate(DIMS):
        hd = D // HEADS
        De, Te = D_EFF[s], T_EFF[s]
        nD = De // 128
        pdim = 128
        ta, tb = rope_tables(TS[s], D, hd)
        if s == 0:
            ta = np.concatenate([ta[:, :Te], ta[:, Te:]], axis=0)  # [128, 1568]
            tb = np.concatenate([tb[:, :Te], tb[:, Te:]], axis=0)
        c[f'ta{s}'] = ta.reshape(nD, pdim, Te).transpose(1, 0, 2).astype(BF16).copy()
        c[f'tb{s}'] = tb.reshape(nD, pdim, Te).transpose(1, 0, 2).astype(BF16).copy()
        if s < 3:
            c[f'mask{s}'] = blockdiag_mask(D_EFF[s] if s == 0 else D, hd)
        for bi in range(DEPTHS[s]):
            fw = fold_block(params['stages'][s][bi], D)
            if s == 0:
                fw = {k: (_stack2(w), np.concatenate([b, b]))
                      for k, (w, b) in fw.items()}
            if s in TS_MODE:
                w, encb, bbc, bsum = wblock_pack_ts(fw, D, ta, tb, TS[s])
                c[f'wts{s}_{bi}'] = w
                c[f'encb{s}_{bi}'] = encb
                c[f'bbc{s}_{bi}'] = bbc
                c[f'bsum{s}_{bi}'] = bsum
            else:
                w, bias = wblock_pack(fw, D_EFF[s])
                c[f'w{s}_{bi}'] = w
                c[f'bias{s}_{bi}'] = bias
        if s in TS_MODE:
            nT = (TS[s] + 127) // 128
            pad = nT * 128 - TS[s]
            tap = np.pad(ta.T, ((0, pad), (0, 0))).reshape(nT, 128, D)
            tbp = np.pad(tb.T, ((0, pad), (0, 0))).reshape(nT, 128, D)
            c[f'tatm{s}'] = tap.transpose(1, 0, 2).astype(BF16).copy()
            c[f'tbtm{s}'] = tbp.transpose(1, 0, 2).astype(BF16).copy()
        if s < 3:
            w2, b2 = fold_conv(params['down'][s])
            cp = conv_pack(w2, DIMS[s], DIMS[s + 1])
            if s == 0:
                cp = np.concatenate([cp, cp], axis=0)  # dup rows for half B
            c[f'conv{s}'] = cp
            c[f'convb{s}'] = b2.reshape(-1, 128).T.astype(np.float32).copy()
    c['ident'] = np.eye(128, dtype=BF16)
    i2 = np.zeros((128, 64), BF16)
    i2[:64] = np.eye(64, dtype=BF16)
    i2[64:] = np.eye(64, dtype=BF16)
    c['ident2'] = i2
    c['ones'] = np.ones((1, 128), dtype=BF16)
    c['magic'] = np.full((128, 32), MAGIC, np.int32)
    return c


# ========================================================== device program

class KB:
    """Kernel builder context."""

    def __init__(self, nc, tc):
        self.nc = nc
        self.tc = tc
        self._rr = 0

    def evac_copy(self, out, in_):
        """Round-robin DVE/ACT for PSUM-source copies (GPSIMD can't read PSUM);
        biased 2:1 toward ACT, which carries less elementwise load."""
        self._rr += 1
        if self._rr % 3 == 0:
            return self.nc.vector.tensor_copy(out=out, in_=in_)
        return self.nc.scalar.copy(out=out, in_=in_)


def build_program():
    nc = bacc.Bacc("TRN2", target_bir_lowering=False, debug=True)
    ins = {}

    def din(name, shape, dt=DT.bfloat16):
        ins[name] = nc.dram_tensor(name, shape, dt, kind="ExternalInput")

    din('xpatch', [96, T_EFF[0]])
    din('patchw', [96, 128])
    din('patchb', [128, 1], DT.float32)
    din('ident', [128, 128])
    din('ident2', [128, 64])
    din('ones', [1, 128])
    din('magic', [128, 32], DT.int32)
    for s, D in enumerate(DIMS):
        De, Te = D_EFF[s], T_EFF[s]
        nD = De // 128
        pdim = min(D, 128)
        din(f'ta{s}', [128, nD, Te])
        din(f'tb{s}', [128, nD, Te])
        if s < 3:
            mp = 128 if s == 0 else pdim
            din(f'mask{s}', [mp, mp])
        wcols = nD * nD * 128 * 6
        for bi in range(DEPTHS[s]):
            if s in TS_MODE:
                nTd = (TS[s] + 127) // 128
                din(f'wts{s}_{bi}', [128, 6 * nD * D])
                din(f'encb{s}_{bi}', [128, nD], DT.float32)
                din(f'bbc{s}_{bi}', [128, 3 * D])
                din(f'bsum{s}_{bi}', [128, nTd * D])
            else:
                din(f'w{s}_{bi}', [128, wcols])
                din(f'bias{s}_{bi}', [128, nD * 6], DT.float32)
        if s in TS_MODE:
            nT = (TS[s] + 127) // 128
            din(f'tatm{s}', [128, nT, D])
            din(f'tbtm{s}', [128, nT, D])
        if s < 3:
            nMo = DIMS[s + 1] // 128
            din(f'conv{s}', [128 if s == 0 else pdim, max(1, D // 128) * 4 * nMo * 128])
            din(f'convb{s}', [128, nMo], DT.float32)

    outs = {s: nc.dram_tensor(f'feat{s}', [T_EFF[s], D_EFF[s]], DT.bfloat16,
                              kind="ExternalOutput")
            for s in range(4)}
    import os
    if os.environ.get('KDBG'):
        for i in range(8):
            outs[f'dbg{i}'] = nc.dram_tensor(f'dbg{i}', [128, 4096], DT.float32,
                                             kind="ExternalOutput")

    with tile.TileContext(nc) as tc:
        _emit(nc, tc, ins, outs)
    nc.compile()
    return nc


def _emit(nc, tc, ins, outs):
    from contextlib import ExitStack
    with ExitStack() as ctx:
        kb = KB(nc, tc)
        kb.consts = ctx.enter_context(tc.tile_pool(name="consts", bufs=1))
        kb.wpool = ctx.enter_context(tc.tile_pool(name="wpool", bufs=2))
        kb.work = ctx.enter_context(tc.tile_pool(name="work", bufs=1))
        kb.resid = ctx.enter_context(tc.tile_pool(name="resid", bufs=2))
        kb.small = ctx.enter_context(tc.tile_pool(name="small", bufs=2))
        kb.ps_mm = ctx.enter_context(tc.tile_pool(name="ps_mm", bufs=4, space="PSUM"))
        kb.ps_tr = ctx.enter_context(tc.tile_pool(name="ps_tr", bufs=2, space="PSUM"))
        kb.ps_s = ctx.enter_context(tc.tile_pool(name="ps_s", bufs=2, space="PSUM"))

        # inputs needed first: patch conv operands (chunked)
        xp = kb.work.tile([96, T_EFF[0]], DT.bfloat16, tag="pk_qv")
        for qi, (t0c, tnc) in enumerate(_chunks(T_EFF[0])):
            nc.sync.dma_start(out=xp[:, t0c:t0c + tnc],
                              in_=ins['xpatch'][:, t0c:t0c + tnc])
        pw = kb.consts.tile([96, 128], DT.bfloat16, tag="patchw")
        nc.sync.dma_start(out=pw, in_=ins['patchw'][:, :])
        pb = kb.consts.tile([128, 1], DT.float32, tag="patchb")
        nc.sync.dma_start(out=pb, in_=ins['patchb'][:, :])
        kb.ident = kb.consts.tile([128, 128], DT.bfloat16)
        nc.sync.dma_start(out=kb.ident, in_=ins['ident'][:, :])
        kb.ident2 = kb.consts.tile([128, 64], DT.bfloat16)
        nc.sync.dma_start(out=kb.ident2, in_=ins['ident2'][:, :])
        kb.ones = kb.consts.tile([1, 128], DT.bfloat16)
        nc.sync.dma_start(out=kb.ones, in_=ins['ones'][:, :])
        kb._dmaq = 0

        def pdma(out, in_):
            nc.sync.dma_start(out=out, in_=in_)
        
        kb.magic = kb.consts.tile([128, 32], DT.int32)
        nc.sync.dma_start(out=kb.magic, in_=ins['magic'][:, :])

        # prefetch every weight/bias/table/mask/conv at kernel start
        kb.wt, kb.bt, kb.ropes, kb.masks, kb.convs = {}, {}, {}, {}, {}
        for s, D in enumerate(DIMS):
            De, Te = D_EFF[s], T_EFF[s]
            nD = De // 128
            pdim = min(D, 128)
            nT = (TS[s] + 127) // 128
            for bi in range(DEPTHS[s]):
                if s in TS_MODE:
                    w = kb.consts.tile([128, nD, 6, D], DT.bfloat16,
                                       tag=f"w{s}_{bi}")
                    pdma(out=w, in_=ins[f'wts{s}_{bi}'][:, :].rearrange(
                        "p (k l d) -> p k l d", l=6, k=nD))
                    eb = kb.consts.tile([128, nD], DT.float32, tag=f"encb{s}_{bi}")
                    pdma(out=eb, in_=ins[f'encb{s}_{bi}'][:, :])
                    bbc = kb.consts.tile([128, 3, D], DT.bfloat16, tag=f"bbc{s}_{bi}")
                    pdma(out=bbc, in_=ins[f'bbc{s}_{bi}'][:, :]
                                      .rearrange("p (l d) -> p l d", l=3))
                    bsum = kb.consts.tile([128, nT, D], DT.bfloat16,
                                          tag=f"bsum{s}_{bi}")
                    pdma(out=bsum, in_=ins[f'bsum{s}_{bi}'][:, :]
                                      .rearrange("p (j d) -> p j d", j=nT))
                    kb.bt[(s, bi)] = (eb, bbc, bsum)
                    kb.wt[(s, bi)] = w
                    continue
                if True:
                    w = kb.consts.tile(list(ins[f'w{s}_{bi}'].shape), DT.bfloat16,
                                       tag=f"w{s}_{bi}")
                    pdma(out=w, in_=ins[f'w{s}_{bi}'][:, :])
                    b = kb.consts.tile(list(ins[f'bias{s}_{bi}'].shape), DT.float32,
                                       tag=f"bias{s}_{bi}")
                    pdma(out=b, in_=ins[f'bias{s}_{bi}'][:, :])
                    kb.wt[(s, bi)], kb.bt[(s, bi)] = w, b
            if s in TS_MODE:
                ta = kb.consts.tile([128, nT, DIMS[s]], DT.bfloat16, tag=f"ta{s}")
                pdma(out=ta, in_=ins[f'tatm{s}'][:, :, :])
                tb = kb.consts.tile([128, nT, DIMS[s]], DT.bfloat16, tag=f"tb{s}")
                pdma(out=tb, in_=ins[f'tbtm{s}'][:, :, :])
            else:
                ta = kb.consts.tile([128, nD, Te], DT.bfloat16, tag=f"ta{s}")
                pdma(out=ta, in_=ins[f'ta{s}'][:, :, :])
                tb = kb.consts.tile([128, nD, Te], DT.bfloat16, tag=f"tb{s}")
                pdma(out=tb, in_=ins[f'tb{s}'][:, :, :])
            kb.ropes[s] = (ta, tb)
            if s < 3:
                mp = 128 if s == 0 else pdim
                mk = kb.consts.tile([mp, mp], DT.bfloat16, tag=f"mask{s}")
                pdma(out=mk, in_=ins[f'mask{s}'][:, :])
                kb.masks[s] = mk
                Cin, Cout = D, DIMS[s + 1]
                nKi, nMo = max(1, Cin // 128), Cout // 128
                wc = kb.consts.tile([128 if s == 0 else min(Cin, 128),
                                     nKi, 4, nMo, 128],
                                    DT.bfloat16, tag=f"conv{s}")
                pdma(out=wc, in_=ins[f'conv{s}'][:, :].rearrange(
                    "p (a b c d) -> p a b c d", a=nKi, b=4, c=nMo))
                cb = kb.consts.tile([128, nMo], DT.float32, tag=f"convb{s}")
                pdma(out=cb, in_=ins[f'convb{s}'][:, :])
                kb.convs[s] = (wc, cb)

        # ---- patch conv (feature-major, stacked) then to token-major
        T0 = T_EFF[0]
        x1_fm = kb.work.tile([128, 1, T0], DT.bfloat16, tag="t2_fm")
        for t0, tn in _chunks(T0):
            mm = kb.ps_mm.tile([128, NCHUNK], DT.float32, tag="mm_ps")
            nc.tensor.matmul(mm[:128, :tn], lhsT=pw, rhs=xp[:, t0:t0 + tn],
                             start=True, stop=True)
            nc.scalar.activation(out=x1_fm[:, 0, t0:t0 + tn], in_=mm[:128, :tn],
                                 func=AF.Identity, bias=pb, scale=1.0)
        x_tm = _fm_to_tm(kb, x1_fm, T0, 128, tag_dst="x_s0", pool='resid',
                         zero_ragged=True)
        import os
        if os.environ.get('KDBG'):
            nc.gpsimd.dma_start(out=outs['dbg0'][:, :64], in_=x_tm[:, 0, :])
            nc.gpsimd.dma_start(out=outs['dbg1'][:64, :512], in_=x1_fm[:, 0, :512])
            nc.gpsimd.dma_start(out=outs['dbg2'][:48, :512], in_=xp[:, :512])
            kb.dbg_outs = outs

        # ---- stages
        for s, (depth, D) in enumerate(zip(DEPTHS, DIMS)):
            for bi in range(depth):
                if s in TS_MODE:
                    x_tm = _block_ts(kb, ins, s, bi, x_tm)
                else:
                    x_tm = _block(kb, ins, s, bi, x_tm)
            for j, psz in enumerate(_tiles(T_EFF[s])):
                nc.sync.dma_start(out=outs[s][j * 128:j * 128 + psz, :],
                                  in_=x_tm[:psz, j, :])
            if s < 3:
                x_tm = _downsample(kb, ins, s, x_tm)


def _fm_to_tm(kb, fm, T, D, tag_dst, pool='work', add_to=None,
              zero_ragged=False):
    """Feature-major [pdim, nD, T] -> token-major [128, nT, D] via packed PE
    transposes. If add_to is given, the grouped evac is a TT add with it
    (residual fusion) routed DVE; else a grouped copy (DVE/ACT)."""
    nc = kb.nc
    pdim = min(D, 128)
    nD = max(1, D // 128)
    sizes = _tiles(T)
    nT = len(sizes)
    nfull = sum(1 for p in sizes if p == 128)
    dst = getattr(kb, pool).tile([128, nT, D], DT.bfloat16, tag=tag_dst)
    npack = max(1, 1024 // pdim)

    for g in range(nD):
        gsl = slice(g * 128, g * 128 + pdim)
        for j0 in range(0, nfull, npack):
            jn = min(npack, nfull - j0)
            pt = kb.ps_tr.tile([128, 1024], DT.bfloat16, tag="tr_ps")
            for k in range(jn):
                nc.tensor.transpose(
                    pt[:, k * pdim:(k + 1) * pdim],
                    fm[:, g, (j0 + k) * 128:(j0 + k + 1) * 128],
                    kb.ident[:pdim, :pdim])
            src = pt[:, :jn * pdim].rearrange("p (j d) -> p j d", j=jn)
            dsl = dst[:, j0:j0 + jn, gsl]
            if add_to is not None:
                nc.vector.tensor_tensor(out=dsl, in0=src,
                                        in1=add_to[:, j0:j0 + jn, gsl], op=OP.add)
            else:
                kb.evac_copy(dsl, src)
        if nfull < nT:  # one ragged tail tile
            j = nT - 1
            psz = sizes[j]
            if zero_ragged:
                nc.gpsimd.memset(dst[:, j, gsl], 0.0)
            pt = kb.ps_tr.tile([128, 1024], DT.bfloat16, tag="tr_ps")
            nc.tensor.transpose(pt[:psz, :pdim], fm[:, g, j * 128:j * 128 + psz],
                                kb.ident[:pdim, :pdim])
            dsl = dst[:psz, j, gsl]
            if add_to is not None:
                nc.vector.tensor_tensor(out=dsl, in0=pt[:psz, :pdim],
                                        in1=add_to[:psz, j, gsl], op=OP.add)
            else:
                kb.evac_copy(dsl, pt[:psz, :pdim])
    return dst


def _tm_to_fm(kb, tm, T, D, tag_dst, evac_fn=None):
    """Token-major [128, nT, D] -> feature-major [pdim, nD, T]."""
    nc = kb.nc
    pdim = min(D, 128)
    nD = max(1, D // 128)
    sizes = _tiles(T)
    nT = len(sizes)
    fm = kb.work.tile([pdim, nD, T], DT.bfloat16, tag=tag_dst)
    npack = 8  # 8 * 128 cols = 1024 bf16 = one bank
    for g in range(nD):
        for j0 in range(0, nT, npack):
            jn = min(npack, nT - j0)
            pt = kb.ps_tr.tile([128, 1024], DT.bfloat16, tag="tr_ps")
            cols = 0
            for k in range(jn):
                j = j0 + k
                psz = sizes[j]
                nc.tensor.transpose(
                    pt[:pdim, cols:cols + psz],
                    tm[:psz, j, g * 128:g * 128 + pdim],
                    kb.ident[:psz, :psz])
                cols += psz
            if evac_fn is not None:
                evac_fn(fm[:, g, j0 * 128:j0 * 128 + cols], pt[:pdim, :cols], g)
            else:
                kb.evac_copy(fm[:, g, j0 * 128:j0 * 128 + cols], pt[:pdim, :cols])
    return fm


def _ln(kb, x_tm, T, D, halves=1):
    """LayerNorm stats + normalize -> xn_tm bf16. halves=2: stage-1 stacked
    layout, each 128-col row holds two tokens (64 cols each)."""
    nc = kb.nc
    sizes = _tiles(T)
    nTt = len(sizes)
    nT = nTt * halves
    Dh = D // halves
    st = kb.small.tile([128, nT, 6], DT.float32, tag="bnst")
    for j in range(nTt):
        for h in range(halves):
            nc.vector.bn_stats(out=st[:, j * halves + h, :],
                               in_=x_tm[:, j, h * Dh:(h + 1) * Dh])
    mv = kb.small.tile([128, nT, 2], DT.float32, tag="mv")
    for j in range(nT):
        nc.vector.bn_aggr(out=mv[:, j, :], in_=st[:, j, :])
    veps = kb.small.tile([128, nT], DT.float32, tag="veps")
    nc.vector.tensor_scalar(out=veps, in0=mv[:, :, 1], scalar1=EPS,
                            scalar2=None, op0=OP.add)
    h = kb.small.tile([128, nT], DT.int32, tag="hshift")
    nc.vector.tensor_scalar(out=h, in0=veps.bitcast(DT.int32), scalar1=1,
                            scalar2=None, op0=OP.logical_shift_right)
    y = kb.small.tile([128, nT], DT.float32, tag="ynewt")
    nc.vector.tensor_tensor(out=y.bitcast(DT.int32), in0=kb.magic[:, :nT],
                            in1=h, op=OP.subtract)
    r = kb.small.tile([128, nT], DT.float32, tag="rnewt")
    for _ in range(NEWTON_ITERS):
        nc.vector.tensor_tensor(out=r, in0=y, in1=y, op=OP.mult)
        nc.vector.tensor_tensor(out=r, in0=r, in1=veps, op=OP.mult)
        nc.vector.tensor_scalar(out=r, in0=r, scalar1=-0.5, scalar2=1.5,
                                op0=OP.mult, op1=OP.add)
        nc.vector.tensor_tensor(out=y, in0=y, in1=r, op=OP.mult)
    xn_tm = kb.work.tile([128, nTt, D], DT.bfloat16, tag="xn_tm")
    for j, psz in enumerate(sizes):
        for h in range(halves):
            jh = j * halves + h
            nc.vector.tensor_scalar(out=xn_tm[:psz, j, h * Dh:(h + 1) * Dh],
                                    in0=x_tm[:psz, j, h * Dh:(h + 1) * Dh],
                                    scalar1=mv[:psz, jh, 0:1],
                                    scalar2=y[:psz, jh:jh + 1],
                                    op0=OP.subtract, op1=OP.mult)
    return xn_tm


def _block(kb, ins, s, bi, x_tm):
    nc = kb.nc
    D, T = D_EFF[s], T_EFF[s]
    pdim = 128
    nD = max(1, D // 128)
    sizes = _tiles(T)
    nT = len(sizes)
    n = nD

    wt, bt = kb.wt[(s, bi)], kb.bt[(s, bi)]
    ta, tb = kb.ropes[s]
    if s < 3:
        kb.mask = kb.masks[s]

    def wsl(li, ki, mo):
        if D == 64:
            off = {0: 0, 2: 128, 4: 256, 5: 320}
            wid = {0: 128, 2: 128, 4: 64, 5: 64}
            return wt[:, off[li]:off[li] + wid[li]]
        base = li * n * n * 128
        return wt[:, base + (ki * n + mo) * 128: base + (ki * n + mo + 1) * 128]

    def bsl(li, mo, p0=0, p1=128):
        col = {0: 0, 2: 1, 3: 4, 4: 2, 5: 3}[li] if D == 64 else mo * 6 + li
        return bt[p0:p1, col:col + 1]

    def linear(li, in_fm, dst_fn):
        """dst_fn(mm_psum, mo, t0, tn) consumes each output chunk."""
        mrows = {0: 128, 2: 128, 4: 64, 5: 64}[li] if D == 64 else pdim
        for mo in range(n):
            for t0, tn in _chunks(T):
                mm = kb.ps_mm.tile([128, NCHUNK], DT.float32, tag="mm_ps")
                for ki in range(n):
                    nc.tensor.matmul(mm[:mrows, :tn], lhsT=wsl(li, ki, mo),
                                     rhs=in_fm[:, ki, t0:t0 + tn],
                                     start=(ki == 0), stop=(ki == n - 1))
                dst_fn(mm, mo, t0, tn)

    # ---- LN, transpose
    xn_tm = _ln(kb, x_tm, T, D, halves=HALVES[s])
    xn_fm = _tm_to_fm(kb, xn_tm, T, D, tag_dst="xn_fm")

    # ---- enc(relu) / gate(sigmoid)  [packed for D=64]
    xl_fm = kb.work.tile([pdim, nD, T], DT.bfloat16, tag="xl_fm")
    gsig_fm = kb.work.tile([pdim, nD, T], DT.bfloat16, tag="gsig_fm")
    if D == 64:
        def encgate_dst(mm, mo, t0, tn):
            nc.scalar.activation(out=xl_fm[:, 0, t0:t0 + tn], in_=mm[0:64, :tn],
                                 func=AF.Relu, bias=bsl(0, 0, 0, 64), scale=1.0)
            nc.scalar.activation(out=gsig_fm[:, 0, t0:t0 + tn], in_=mm[64:128, :tn],
                                 func=AF.Sigmoid, bias=bsl(0, 0, 64, 128), scale=1.0)
        linear(0, xn_fm, encgate_dst)
    else:
        def enc_dst(mm, mo, t0, tn):
            nc.scalar.activation(out=xl_fm[:, mo, t0:t0 + tn], in_=mm[:pdim, :tn],
                                 func=AF.Relu, bias=bsl(0, mo), scale=1.0)
        linear(0, xn_fm, enc_dst)

        def gate_dst(mm, mo, t0, tn):
            nc.scalar.activation(out=gsig_fm[:, mo, t0:t0 + tn], in_=mm[:pdim, :tn],
                                 func=AF.Sigmoid, bias=bsl(1, mo), scale=1.0)
        linear(1, xn_fm, gate_dst)

    # ---- qk/qks with fused bias+rope-mult; v with bias
    t1_fm = kb.work.tile([pdim, nD, T], DT.bfloat16, tag="t1_fm")
    t2_fm = kb.work.tile([pdim, nD, T], DT.bfloat16, tag="t2_fm")
    if D == 64:
        def qkqks_dst(mm, mo, t0, tn):
            nc.vector.scalar_tensor_tensor(
                out=t1_fm[:, 0, t0:t0 + tn], in0=mm[0:64, :tn],
                scalar=bsl(2, 0, 0, 64), in1=ta[:, 0, t0:t0 + tn],
                op0=OP.add, op1=OP.mult)
            nc.vector.scalar_tensor_tensor(
                out=t2_fm[:, 0, t0:t0 + tn], in0=mm[64:128, :tn],
                scalar=bsl(3, 0, 0, 64), in1=tb[:, 0, t0:t0 + tn],
                op0=OP.add, op1=OP.mult)
        linear(2, xl_fm, qkqks_dst)
    else:
        def qk_dst(mm, mo, t0, tn):
            nc.vector.scalar_tensor_tensor(
                out=t1_fm[:, mo, t0:t0 + tn], in0=mm[:pdim, :tn],
                scalar=bsl(2, mo), in1=ta[:, mo, t0:t0 + tn],
                op0=OP.add, op1=OP.mult)
        linear(2, xl_fm, qk_dst)

        def qks_dst(mm, mo, t0, tn):
            nc.vector.scalar_tensor_tensor(
                out=t2_fm[:, mo, t0:t0 + tn], in0=mm[:pdim, :tn],
                scalar=bsl(3, mo), in1=tb[:, mo, t0:t0 + tn],
                op0=OP.add, op1=OP.mult)
        linear(3, xl_fm, qks_dst)

    if D == 64:
        # pack qkr (rows 0:64) and v (rows 64:128) into one tile: one
        # transpose covers both for the S stage.
        pk = kb.work.tile([128, 1, T], DT.bfloat16, tag="pk_qv")

        def v_dst(mm, mo, t0, tn):
            nc.scalar.activation(out=pk[64:128, 0, t0:t0 + tn], in_=mm[0:64, :tn],
                                 func=AF.Identity, bias=bsl(4, 0, 0, 64), scale=1.0)
        linear(4, xl_fm, v_dst)
        nc.vector.tensor_tensor(out=pk[0:64, 0, :], in0=t1_fm[:, 0, :],
                                in1=t2_fm[:, 0, :], op=OP.add)
        qv_tm = _fm_to_tm(kb, pk, T, 128, tag_dst="xn_tm")
        qkr_fm = pk
    else:
        v_fm = kb.work.tile([pdim, nD, T], DT.bfloat16, tag="v_fm")

        def v_dst(mm, mo, t0, tn):
            nc.scalar.activation(out=v_fm[:, mo, t0:t0 + tn], in_=mm[:pdim, :tn],
                                 func=AF.Identity, bias=bsl(4, mo, 0, pdim), scale=1.0)
        linear(4, xl_fm, v_dst)

        qkr_fm = kb.work.tile([pdim, nD, T], DT.bfloat16, tag="qkr_fm")
        for g in range(nD):
            eng = nc.vector if g % 2 == 0 else nc.gpsimd
            eng.tensor_tensor(out=qkr_fm[:, g, :], in0=t1_fm[:, g, :],
                              in1=t2_fm[:, g, :], op=OP.add)
        qkr_tm = _fm_to_tm(kb, qkr_fm, T, D, tag_dst="qkr_tm")
        v_tm = _fm_to_tm(kb, v_fm, T, D, tag_dst="v_tm")

    # ---- S per feature group + attn
    attn_fm = kb.work.tile([pdim, nD, T], DT.bfloat16, tag="t1_fm")
    for g in range(nD):
        sp = kb.ps_s.tile([128, 128], DT.float32, tag="s_ps")
        for j, psz in enumerate(sizes):
            if D == 64:
                lhsT = qv_tm[:psz, j, 0:64]
                rhs = qv_tm[:psz, j, 64:128]
            else:
                lhsT = qkr_tm[:psz, j, g * 128:g * 128 + pdim]
                rhs = v_tm[:psz, j, g * 128:g * 128 + pdim]
            nc.tensor.matmul(sp[:pdim, :pdim], lhsT=lhsT, rhs=rhs,
                             start=(j == 0), stop=(j == nT - 1))
        sbd = kb.small.tile([128, 128], DT.bfloat16, tag="sbd")
        if s == 0:
            # stacked: S_full = sum of the two diagonal 64-blocks of sp
            ssb = kb.small.tile([128, 128], DT.bfloat16, tag="ssb")
            nc.vector.tensor_tensor(out=ssb, in0=sp, in1=kb.mask, op=OP.mult)
            rp = kb.ps_s.tile([128, 128], DT.float32, tag="s_ps")
            nc.tensor.matmul(rp[:64, :128], lhsT=kb.ident2, rhs=ssb,
                             start=True, stop=True)
            rs = kb.small.tile([64, 128], DT.bfloat16, tag="rssb")
            kb.evac_copy(rs, rp[:64, :128])
            sfull = kb.small.tile([64, 64], DT.bfloat16, tag="sfull")
            nc.vector.tensor_tensor(out=sfull, in0=rs[:, 0:64], in1=rs[:, 64:128],
                                    op=OP.add)
            nc.vector.memset(sbd, 0.0)
            nc.vector.tensor_copy(out=sbd[0:64, 0:64], in_=sfull)
            nc.vector.tensor_copy(out=sbd[64:128, 64:128], in_=sfull)
        elif s < 3:
            nc.vector.tensor_tensor(out=sbd[:pdim, :pdim], in0=sp[:pdim, :pdim],
                                    in1=kb.mask, op=OP.mult)
        else:
            nc.vector.tensor_copy(out=sbd[:pdim, :pdim], in_=sp[:pdim, :pdim])
        for t0, tn in _chunks(T):
            ap = kb.ps_mm.tile([128, NCHUNK], DT.float32, tag="mm_ps")
            nc.tensor.matmul(ap[:pdim, :tn], lhsT=sbd[:pdim, :pdim],
                             rhs=qkr_fm[0:pdim, g, t0:t0 + tn], start=True, stop=True)
            kb.evac_copy(attn_fm[:, g, t0:t0 + tn], ap[:pdim, :tn])

    # ---- out proj: u = (psum + b) * gsig
    u_fm = kb.work.tile([pdim, nD, T], DT.bfloat16, tag="t2_fm")

    def out_dst(mm, mo, t0, tn):
        nc.vector.scalar_tensor_tensor(
            out=u_fm[:, mo, t0:t0 + tn], in0=mm[:pdim, :tn],
            scalar=bsl(5, mo, 0, pdim), in1=gsig_fm[:, mo, t0:t0 + tn],
            op0=OP.add, op1=OP.mult)
    linear(5, attn_fm, out_dst)

    # ---- residual: x_new = x + u^T (fused into transpose evac)
    x_new = _fm_to_tm(kb, u_fm, T, D, tag_dst=f"x_s{s}", pool='resid',
                      add_to=x_tm, zero_ragged=True)
    import os
    if os.environ.get('KDBG') and s == 0 and bi == 0:
        o = kb.dbg_outs
        nc.gpsimd.dma_start(out=o['dbg3'][:, :64], in_=xn_tm[:, 0, :])
        nc.gpsimd.dma_start(out=o['dbg3'][:, 64:128], in_=xn_tm[:, 1, :])
        nc.gpsimd.dma_start(out=o['dbg3'][:, 128:192], in_=x_tm[:, 1, :])
        nc.gpsimd.dma_start(out=o['dbg3'][:, 192:256], in_=x_tm[:, 2, :])
        nc.gpsimd.dma_start(out=o['dbg4'][:64, :512], in_=xl_fm[:, 0, :512])
        nc.gpsimd.dma_start(out=o['dbg4'][64:128, :512], in_=xn_fm[:, 0, :512])
        nc.gpsimd.dma_start(out=o['dbg5'][:64, :512], in_=qkr_fm[:, 0, :512])
        nc.gpsimd.dma_start(out=o['dbg6'][:64, :512], in_=attn_fm[:, 0, :512])
        nc.gpsimd.dma_start(out=o['dbg7'][:64, :512], in_=u_fm[:, 0, :512])
    return x_new




def _block_ts(kb, ins, s, bi, x_tm):
    """Token-stationary block for small-T stages: activations as lhsT,
    weights as moving rhs, most tensors token-major."""
    nc = kb.nc
    D, T = DIMS[s], TS[s]
    nK = D // 128
    sizes = _tiles(T)
    nT = len(sizes)
    wt = kb.wt[(s, bi)]                  # [128, 6, nK, D]
    encb, bbc, bsum = kb.bt[(s, bi)]     # [128,nK] f32, [128,3,D], [128,nT,D]
    ta, tb = kb.ropes[s]                 # token-major [128, nT, D]

    xn_tm = _ln(kb, x_tm, T, D)
    xn_fm = _tm_to_fm(kb, xn_tm, T, D, tag_dst="xn_fm")

    def linear_grp(grp, in_fm, dst_fn):
        gw = len(grp) * D
        for j, psz in enumerate(sizes):
            mm = kb.ps_mm.tile([128, NCHUNK], DT.float32, tag="mm_ps")
            for ki in range(nK):
                nc.tensor.matmul(mm[:psz, :gw],
                                 lhsT=in_fm[:, ki, j * 128:j * 128 + psz],
                                 rhs=wt[:, ki, grp[0]:grp[0] + len(grp), :],
                                 start=(ki == 0), stop=(ki == nK - 1))
            dst_fn(mm, j, psz)

    pair = (D <= 256)

    # enc(+gate): psum -> xlp_tm (copy) -> transpose with fused Relu+bias evac
    xlp_tm = kb.work.tile([128, nT, D], DT.bfloat16, tag="xl_tm")
    gsig_tm = kb.work.tile([128, nT, D], DT.bfloat16, tag="gsig_tm")

    def gate_cols(mm, j, psz, c0):
        nc.vector.tensor_tensor(out=gsig_tm[:psz, j, :], in0=mm[:psz, c0:c0 + D],
                                in1=bbc[:psz, 0, :], op=OP.add)
        nc.scalar.activation(out=gsig_tm[:psz, j, :], in_=gsig_tm[:psz, j, :],
                             func=AF.Sigmoid)

    if pair:
        def encgate_dst(mm, j, psz):
            kb.evac_copy(xlp_tm[:psz, j, :], mm[:psz, 0:D])
            gate_cols(mm, j, psz, D)
        linear_grp([0, 1], xn_fm, encgate_dst)
    else:
        linear_grp([0], xn_fm, lambda mm, j, psz: kb.evac_copy(
            xlp_tm[:psz, j, :], mm[:psz, :D]))
        linear_grp([1], xn_fm, lambda mm, j, psz: gate_cols(mm, j, psz, 0))
    xl_fm = _tm_to_fm(kb, xlp_tm, T, D, tag_dst="xl_fm",
                      evac_fn=lambda dsl, srcp, g: nc.scalar.activation(
                          out=dsl, in_=srcp, func=AF.Relu,
                          bias=encb[:, g:g + 1], scale=1.0))

    # qk/qks: psum * table -> t1/t2 ; v: psum + bias_bc
    t1_tm = kb.work.tile([128, nT, D], DT.bfloat16, tag="t1_tm")
    t2_tm = kb.work.tile([128, nT, D], DT.bfloat16, tag="t2_tm")
    if pair:
        def qkqks_dst(mm, j, psz):
            nc.vector.tensor_tensor(out=t1_tm[:psz, j, :], in0=mm[:psz, 0:D],
                                    in1=ta[:psz, j, :], op=OP.mult)
            nc.vector.tensor_tensor(out=t2_tm[:psz, j, :], in0=mm[:psz, D:2 * D],
                                    in1=tb[:psz, j, :], op=OP.mult)
        linear_grp([2, 3], xl_fm, qkqks_dst)
    else:
        linear_grp([2], xl_fm, lambda mm, j, psz: nc.vector.tensor_tensor(
            out=t1_tm[:psz, j, :], in0=mm[:psz, :D], in1=ta[:psz, j, :],
            op=OP.mult))
        linear_grp([3], xl_fm, lambda mm, j, psz: nc.vector.tensor_tensor(
            out=t2_tm[:psz, j, :], in0=mm[:psz, :D], in1=tb[:psz, j, :],
            op=OP.mult))
    v_tm = kb.work.tile([128, nT, D], DT.bfloat16, tag="v_tm")
    linear_grp([4], xl_fm, lambda mm, j, psz: nc.vector.tensor_tensor(
        out=v_tm[:psz, j, :], in0=mm[:psz, :D], in1=bbc[:psz, 1, :],
        op=OP.add))
    # qkr = t1 + t2 + bsum
    qkr_tm = kb.work.tile([128, nT, D], DT.bfloat16, tag="qkr_tm")
    for j, psz in enumerate(sizes):
        nc.vector.tensor_tensor(out=qkr_tm[:psz, j, :], in0=t1_tm[:psz, j, :],
                                in1=t2_tm[:psz, j, :], op=OP.add)
        nc.gpsimd.tensor_tensor(out=qkr_tm[:psz, j, :], in0=qkr_tm[:psz, j, :],
                                in1=bsum[:psz, j, :], op=OP.add)

    qkr_fm = _tm_to_fm(kb, qkr_tm, T, D, tag_dst="xl_fm")

    # S + attn (fm result)
    attn_fm = kb.work.tile([128, nK, T], DT.bfloat16, tag="t1_tm")
    for g in range(nK):
        sp = kb.ps_s.tile([128, 128], DT.float32, tag="s_ps")
        for j, psz in enumerate(sizes):
            nc.tensor.matmul(sp, lhsT=qkr_tm[:psz, j, g * 128:g * 128 + 128],
                             rhs=v_tm[:psz, j, g * 128:g * 128 + 128],
                             start=(j == 0), stop=(j == nT - 1))
        sbd = kb.small.tile([128, 128], DT.bfloat16, tag="sbd")
        if s < 3:
            nc.vector.tensor_tensor(out=sbd, in0=sp, in1=kb.masks[s], op=OP.mult)
        else:
            nc.vector.tensor_copy(out=sbd, in_=sp)
        ap = kb.ps_mm.tile([128, NCHUNK], DT.float32, tag="mm_ps")
        nc.tensor.matmul(ap[:128, :T], lhsT=sbd, rhs=qkr_fm[:, g, :],
                         start=True, stop=True)
        kb.evac_copy(attn_fm[:, g, :], ap[:128, :T])

    # out proj + bias + gate + residual
    x_new = kb.resid.tile([128, nT, D], DT.bfloat16, tag=f"x_s{s}")
    u_tm = kb.work.tile([128, nT, D], DT.bfloat16, tag="u_tm")

    def out_dst(mm, j, psz):
        if psz < 128:
            nc.gpsimd.memset(x_new[:, j, :], 0.0)
        nc.vector.tensor_tensor(out=u_tm[:psz, j, :], in0=mm[:psz, :D],
                                in1=bbc[:psz, 2, :], op=OP.add)
        nc.vector.tensor_tensor(out=u_tm[:psz, j, :], in0=u_tm[:psz, j, :],
                                in1=gsig_tm[:psz, j, :], op=OP.mult)
        nc.gpsimd.tensor_tensor(out=x_new[:psz, j, :], in0=u_tm[:psz, j, :],
                                in1=x_tm[:psz, j, :], op=OP.add)
    linear_grp([5], attn_fm, out_dst)
    return x_new


def _downsample(kb, ins, s, x_tm):
    nc = kb.nc
    Cin, Cout = DIMS[s], DIMS[s + 1]
    T, Tn = TS[s], TS[s + 1]
    H, Ho = HWS[s], HWS[s + 1]
    pi = min(Cin, 128)
    nKi, nMo = max(1, Cin // 128), Cout // 128

    wc, cb = kb.convs[s]

    if s == 0:
        x_fm = _tm_to_fm(kb, x_tm, T_EFF[0], 128, tag_dst="gsig_fm")
        y_fm = kb.work.tile([128, nMo, Tn], DT.bfloat16, tag="t2_fm")
        for mo in range(nMo):
            for h in range(2):
                mm = kb.ps_mm.tile([128, NCHUNK], DT.float32, tag="mm_ps")
                imv = x_fm[h * 64:(h + 1) * 64, 0, :].rearrange(
                    "p (ho a wo b) -> p ho a wo b", a=2, b=2, ho=14)
                for k, (dy, dx) in enumerate([(0, 0), (0, 1), (1, 0), (1, 1)]):
                    nc.tensor.matmul(mm[:128, :392],
                                     lhsT=wc[h * 64:(h + 1) * 64, 0, k, mo, :],
                                     rhs=imv[:, :, dy, :, dx],
                                     start=(k == 0), stop=(k == 3))
                nc.scalar.activation(out=y_fm[:, mo, h * 392:(h + 1) * 392],
                                     in_=mm[:128, :392], func=AF.Identity,
                                     bias=cb[:, mo:mo + 1], scale=1.0)
        return _fm_to_tm(kb, y_fm, Tn, Cout, tag_dst="x_s1", pool='resid',
                         zero_ragged=True)

    x_fm = _tm_to_fm(kb, x_tm, T, Cin, tag_dst="gsig_fm")
    y_fm = kb.work.tile([128, nMo, Tn], DT.bfloat16, tag="t2_fm")

    rows = max(1, NCHUNK // Ho)
    for mo in range(nMo):
        for y0 in range(0, Ho, rows):
            yn = min(rows, Ho - y0)
            mm = kb.ps_mm.tile([128, NCHUNK], DT.float32, tag="mm_ps")
            first = True
            for ki in range(nKi):
                imv = x_fm[:, ki, :].rearrange("p (ho a wo b) -> p ho a wo b",
                                               a=2, b=2, ho=H // 2)
                for k, (dy, dx) in enumerate([(0, 0), (0, 1), (1, 0), (1, 1)]):
                    nc.tensor.matmul(mm[:128, :yn * Ho], lhsT=wc[:, ki, k, mo, :],
                                     rhs=imv[:, y0:y0 + yn, dy, :, dx],
                                     start=first, stop=(ki == nKi - 1 and k == 3))
                    first = False
            nc.scalar.activation(out=y_fm[:, mo, y0 * Ho:(y0 + yn) * Ho],
                                 in_=mm[:128, :yn * Ho], func=AF.Identity,
                                 bias=cb[:, mo:mo + 1], scale=1.0)

    return _fm_to_tm(kb, y_fm, Tn, Cout, tag_dst=f"x_s{s + 1}", pool='resid',
                     zero_ragged=True)


# ========================================================== public entry

_CACHE = {}


def _get_program():
    if 'nc' not in _CACHE:
        _CACHE['nc'] = build_program()
    return _CACHE['nc']


def make_in_maps(x, params):
    x = np.asarray(x, dtype=np.float32)
    consts = prep_consts(params)
    in_maps = []
    for core in range(8):
        m = dict(consts)
        m['xpatch'] = im2col_patch(x[core % B]).astype(BF16)
        in_maps.append(m)
    return in_maps


def assemble(results):
    feats = []
    for s, D in enumerate(DIMS):
        H = HWS[s]
        imgs = []
        for b in range(B):
            a = np.asarray(results[b][f'feat{s}']).astype(np.float32)
            if s == 0:
                a = np.concatenate([a[:, :64], a[:, 64:]], axis=0)
            imgs.append(a.T.reshape(D, H, H))
        feats.append(np.stack(imgs))
    return tuple(feats)


def kernel(x, params):
    from concourse.bass_utils import run_bass_kernel_spmd
    nc = _get_program()
    in_maps = make_in_maps(x, params)
    res = run_bass_kernel_spmd(nc, in_maps, list(range(8)))
    return assemble(res.results)


# revision 48
# speedup vs baseline: 1.0079x; 1.0059x over previous
"""Trainium2 Bass kernel for nn_BDHEncoder (hierarchical vision transformer).

Key ideas:
- Linear attention: (qk qk^T) v == qk (qk^T v)  (no softmax in reference),
  so attention is O(N*hd^2) instead of O(N^2*hd).
- Data-parallel over the 4 images (cores 0-3; 4-7 duplicates).
- Residual kept token-major [128, nT, D] (cheap per-token LN via bn_stats +
  Newton-rsqrt, no ACT table); activations feature-major [128, nD, T] for
  weight-stationary matmuls; PE transposes bridge, packed 8-16 per PSUM bank
  with one grouped evacuation each.
- RoPE via 2 host tables; the even/odd de-interleave + half-swap live in the
  qk weight rows (second "swapped" qk matmul), 1/sqrt(hd) folded into tables.
- LN gamma/beta folded into enc/gate weights; BN folded into convs; biases
  fused into PSUM-evac ops (ACT activation bias / DVE scalar_tensor_tensor).
- Only ACT table function used is Sigmoid -> single table load.
- bf16 everywhere, f32 PSUM/stats.

Self-contained: hardcodes all shapes from the problem spec.
"""
import sys

if '/opt/trn_rl_repo' not in sys.path:
    sys.path.insert(0, '/opt/trn_rl_repo')

import numpy as np
import ml_dtypes

import concourse.bass as bass
import concourse.mybir as mybir
import concourse.tile as tile
from concourse import bacc

BF16 = ml_dtypes.bfloat16
DT = mybir.dt
OP = mybir.AluOpType
AF = mybir.ActivationFunctionType

DEPTHS = [2, 2, 4, 2]
DIMS = [64, 128, 256, 512]
HEADS = 4
EPS = 1e-5
HWS = [56, 28, 14, 7]
TS = [h * h for h in HWS]
B = 4
# stage 1 runs "stacked": tokens folded in half onto the partition axis,
# so it looks like a D=128, T=1568 stage with block-diagonal weights.
D_EFF = [128, 128, 256, 512]
T_EFF = [1568, 784, 196, 49]
HALVES = [2, 1, 1, 1]
NCHUNK = 512
MAGIC = 0x5F3759DF
NEWTON_ITERS = 2
LAYERS = ['enc', 'gate', 'qk', 'qks', 'v', 'out']


def _tiles(T):
    return [min(128, T - j * 128) for j in range((T + 127) // 128)]


def _chunks(T, c=NCHUNK):
    return [(t0, min(c, T - t0)) for t0 in range(0, T, c)]


# ============================================================ host-side prep

def f32(a):
    return np.asarray(a, dtype=np.float32)


def _rope_perm(D, hd):
    p = []
    for h in range(D // hd):
        p += [h * hd + i for i in range(0, hd, 2)]
        p += [h * hd + i for i in range(1, hd, 2)]
    return np.array(p)


def _swap_perm(D, hd):
    p = []
    for h in range(D // hd):
        p += list(range(h * hd + hd // 2, h * hd + hd))
        p += list(range(h * hd, h * hd + hd // 2))
    return np.array(p)


def rope_tables(T, D, hd):
    inv_freq = 1.0 / (10000.0 ** (np.arange(0, D, 2, dtype=np.float64) / D))
    freqs = np.arange(T, dtype=np.float64)[:, None] * inv_freq[None, :]
    base = np.cos(freqs[:, :hd]).astype(np.float32)
    cos_t, sin_t = np.cos(base), np.sin(base)
    A, Bb = cos_t[:, 0::2], sin_t[:, 0::2]
    C, Dd = sin_t[:, 1::2], cos_t[:, 1::2]
    s = float(hd) ** -0.25
    ta = np.tile(np.concatenate([A, Dd], 1).T * s, (D // hd, 1))
    tb = np.tile(np.concatenate([-Bb, C], 1).T * s, (D // hd, 1))
    return ta.astype(np.float32), tb.astype(np.float32)


def fold_block(p, D):
    g, b = f32(p['ln_g']), f32(p['ln_b'])
    hd = D // HEADS
    pr, sw = _rope_perm(D, hd), _swap_perm(D, hd)
    enc_w, gate_w = f32(p['enc_w']), f32(p['gate_w'])
    qk_w, v_w, out_w = f32(p['qk_w']), f32(p['v_w']), f32(p['out_w'])
    return dict(
        enc=((enc_w * g[None, :]).T, f32(p['enc_b']) + enc_w @ b),
        gate=((gate_w * g[None, :]).T, f32(p['gate_b']) + gate_w @ b),
        qk=(qk_w[pr].T, f32(p['qk_b'])[pr]),
        qks=(qk_w[pr][sw].T, f32(p['qk_b'])[pr][sw]),
        v=(v_w.T, f32(p['v_b'])),
        out=(out_w.T, f32(p['out_b'])))


def wblock_pack(fw, D):
    """Pack block weights -> bf16 [pdim, cols], biases -> f32 [128, bcols]."""
    if D == 64:
        w = np.concatenate([
            np.concatenate([fw['enc'][0], fw['gate'][0]], 1),
            np.concatenate([fw['qk'][0], fw['qks'][0]], 1),
            fw['v'][0], fw['out'][0]], axis=1)      # [64, 384]
        bias = np.zeros((128, 5), np.float32)
        bias[:, 0] = np.concatenate([fw['enc'][1], fw['gate'][1]])
        bias[:, 1] = np.concatenate([fw['qk'][1], fw['qks'][1]])
        bias[:64, 2] = fw['v'][1]
        bias[:64, 3] = fw['out'][1]
        bias[:64, 4] = fw['qks'][1]
        return w.astype(BF16), bias
    n = D // 128
    mats, bias = [], np.zeros((128, n, 6), np.float32)
    for li, l in enumerate(LAYERS):
        W, bv = fw[l]
        mats.append(W.reshape(n, 128, n, 128).transpose(1, 0, 2, 3).reshape(128, -1))
        bias[:, :, li] = bv.reshape(n, 128).T
    return np.concatenate(mats, 1).astype(BF16), bias.reshape(128, -1)


def fold_conv(d):
    w, b = f32(d['conv_w']), f32(d['conv_b'])
    inv = f32(d['bn_g']) / np.sqrt(f32(d['bn_v']) + EPS)
    return (w * inv[:, None, None, None],
            b * inv + f32(d['bn_b']) - f32(d['bn_m']) * inv)


def conv_pack(w2, Cin, Cout):
    nKi, nMo = max(1, Cin // 128), Cout // 128
    pi = min(Cin, 128)
    out = np.zeros((pi, nKi, 4, nMo, 128), np.float32)
    for dy in range(2):
        for dx in range(2):
            wt = w2[:, :, dy, dx].T
            out[:, :, dy * 2 + dx, :, :] = \
                wt.reshape(nKi, pi, nMo, 128).transpose(1, 0, 2, 3)
    return out.reshape(pi, -1).astype(BF16)


def im2col_patch(x_img):
    c = np.asarray(x_img, np.float32).reshape(3, 56, 4, 56, 4)
    p = c.transpose(0, 2, 4, 1, 3).reshape(48, 3136)
    return np.concatenate([p[:, :1568], p[:, 1568:]], axis=0)  # [96, 1568]


def blockdiag_mask(D, hd):
    g = min(D, 128)
    m = np.zeros((g, g), np.float32)
    for h0 in range(0, g, hd):
        m[h0:h0 + hd, h0:h0 + hd] = 1.0
    return m.astype(BF16)


TS_MODE = (3,)   # stages using token-stationary matmuls


def wblock_pack_ts(fw, D, ta, tb, T):
    """ts-mode: weights as moving rhs [128, L, nKi, D]; biases as:
    encb [128, nK] f32 (per-partition, applied at transpose evac);
    bbc [128, 3, D] bf16 broadcast rows (gate, v, out);
    bsum [128, nT, D] bf16 = qk_b*ta + qks_b*tb (token-major)."""
    n = D // 128
    mats = np.zeros((128, n, 6, D), np.float32)
    for li, l in enumerate(LAYERS):
        mats[:, :, li, :] = fw[l][0].reshape(n, 128, D).transpose(1, 0, 2)
    encb = fw['enc'][1].reshape(n, 128).T.astype(np.float32).copy()
    bbc = np.zeros((128, 3, D), np.float32)
    bbc[:, 0, :] = fw['gate'][1][None, :]
    bbc[:, 1, :] = fw['v'][1][None, :]
    bbc[:, 2, :] = fw['out'][1][None, :]
    bs = fw['qk'][1][None, :] * ta.T + fw['qks'][1][None, :] * tb.T  # [T, D]
    nT = (T + 127) // 128
    bs = np.pad(bs, ((0, nT * 128 - T), (0, 0))).reshape(nT, 128, D)
    bsum = bs.transpose(1, 0, 2)
    return (mats.reshape(128, -1).astype(BF16), encb,
            bbc.reshape(128, -1).astype(BF16),
            bsum.reshape(128, -1).astype(BF16).copy())


def _stack2(a):
    """block-diag stack of a [n, m] -> [2n, 2m]"""
    n, m = a.shape
    o = np.zeros((2 * n, 2 * m), a.dtype)
    o[:n, :m] = a
    o[n:, m:] = a
    return o


def prep_consts(params):
    c = {}
    pw = f32(params['patch_w']).reshape(64, 48).T    # [48, 64]
    c['patchw'] = _stack2(pw).astype(BF16).copy()    # [96, 128]
    pb = f32(params['patch_b'])
    c['patchb'] = np.concatenate([pb, pb]).reshape(128, 1).copy()
    for s, D in enumerate(DIMS):
        hd = D // HEADS
        De, Te = D_EFF[s], T_EFF[s]
        nD = De // 128
        pdim = 128
        ta, tb = rope_tables(TS[s], D, hd)
        if s == 0:
            ta = np.concatenate([ta[:, :Te], ta[:, Te:]], axis=0)  # [128, 1568]
            tb = np.concatenate([tb[:, :Te], tb[:, Te:]], axis=0)
        c[f'ta{s}'] = ta.reshape(nD, pdim, Te).transpose(1, 0, 2).astype(BF16).copy()
        c[f'tb{s}'] = tb.reshape(nD, pdim, Te).transpose(1, 0, 2).astype(BF16).copy()
        if s < 3:
            c[f'mask{s}'] = blockdiag_mask(D_EFF[s] if s == 0 else D, hd)
        for bi in range(DEPTHS[s]):
            fw = fold_block(params['stages'][s][bi], D)
            if s == 0:
                fw = {k: (_stack2(w), np.concatenate([b, b]))
                      for k, (w, b) in fw.items()}
            if s in TS_MODE:
                w, encb, bbc, bsum = wblock_pack_ts(fw, D, ta, tb, TS[s])
                c[f'wts{s}_{bi}'] = w
                c[f'encb{s}_{bi}'] = encb
                c[f'bbc{s}_{bi}'] = bbc
                c[f'bsum{s}_{bi}'] = bsum
            else:
                w, bias = wblock_pack(fw, D_EFF[s])
                c[f'w{s}_{bi}'] = w
                c[f'bias{s}_{bi}'] = bias
        if s in TS_MODE:
            nT = (TS[s] + 127) // 128
            pad = nT * 128 - TS[s]
            tap = np.pad(ta.T, ((0, pad), (0, 0))).reshape(nT, 128, D)
            tbp = np.pad(tb.T, ((0, pad), (0, 0))).reshape(nT, 128, D)
            c[f'tatm{s}'] = tap.transpose(1, 0, 2).astype(BF16).copy()
            c[f'tbtm{s}'] = tbp.transpose(1, 0, 2).astype(BF16).copy()
        if s < 3:
            w2, b2 = fold_conv(params['down'][s])
            cp = conv_pack(w2, DIMS[s], DIMS[s + 1])
            if s == 0:
                cp = np.concatenate([cp, cp], axis=0)  # dup rows for half B
            c[f'conv{s}'] = cp
            c[f'convb{s}'] = b2.reshape(-1, 128).T.astype(np.float32).copy()
    c['ident'] = np.eye(128, dtype=BF16)
    i2 = np.zeros((128, 64), BF16)
    i2[:64] = np.eye(64, dtype=BF16)
    i2[64:] = np.eye(64, dtype=BF16)
    c['ident2'] = i2
    c['ones'] = np.ones((1, 128), dtype=BF16)
    c['magic'] = np.full((128, 32), MAGIC, np.int32)
    return c


# ========================================================== device program

class KB:
    """Kernel builder context."""

    def __init__(self, nc, tc):
        self.nc = nc
        self.tc = tc
        self._rr = 0

    def evac_copy(self, out, in_):
        """Round-robin DVE/ACT for PSUM-source copies (GPSIMD can't read PSUM);
        biased 2:1 toward ACT, which carries less elementwise load."""
        self._rr += 1
        if self._rr % 3 == 0:
            return self.nc.vector.tensor_copy(out=out, in_=in_)
        return self.nc.scalar.copy(out=out, in_=in_)


def build_program():
    nc = bacc.Bacc("TRN2", target_bir_lowering=False, debug=True)
    ins = {}

    def din(name, shape, dt=DT.bfloat16):
        ins[name] = nc.dram_tensor(name, shape, dt, kind="ExternalInput")

    din('xpatch', [96, T_EFF[0]])
    din('patchw', [96, 128])
    din('patchb', [128, 1], DT.float32)
    din('ident', [128, 128])
    din('ident2', [128, 64])
    din('ones', [1, 128])
    din('magic', [128, 32], DT.int32)
    for s, D in enumerate(DIMS):
        De, Te = D_EFF[s], T_EFF[s]
        nD = De // 128
        pdim = min(D, 128)
        din(f'ta{s}', [128, nD, Te])
        din(f'tb{s}', [128, nD, Te])
        if s < 3:
            mp = 128 if s == 0 else pdim
            din(f'mask{s}', [mp, mp])
        wcols = nD * nD * 128 * 6
        for bi in range(DEPTHS[s]):
            if s in TS_MODE:
                nTd = (TS[s] + 127) // 128
                din(f'wts{s}_{bi}', [128, 6 * nD * D])
                din(f'encb{s}_{bi}', [128, nD], DT.float32)
                din(f'bbc{s}_{bi}', [128, 3 * D])
                din(f'bsum{s}_{bi}', [128, nTd * D])
            else:
                din(f'w{s}_{bi}', [128, wcols])
                din(f'bias{s}_{bi}', [128, nD * 6], DT.float32)
        if s in TS_MODE:
            nT = (TS[s] + 127) // 128
            din(f'tatm{s}', [128, nT, D])
            din(f'tbtm{s}', [128, nT, D])
        if s < 3:
            nMo = DIMS[s + 1] // 128
            din(f'conv{s}', [128 if s == 0 else pdim, max(1, D // 128) * 4 * nMo * 128])
            din(f'convb{s}', [128, nMo], DT.float32)

    outs = {s: nc.dram_tensor(f'feat{s}', [T_EFF[s], D_EFF[s]], DT.bfloat16,
                              kind="ExternalOutput")
            for s in range(4)}
    import os
    if os.environ.get('KDBG'):
        for i in range(8):
            outs[f'dbg{i}'] = nc.dram_tensor(f'dbg{i}', [128, 4096], DT.float32,
                                             kind="ExternalOutput")

    with tile.TileContext(nc) as tc:
        _emit(nc, tc, ins, outs)
    nc.compile()
    return nc


def _emit(nc, tc, ins, outs):
    from contextlib import ExitStack
    with ExitStack() as ctx:
        kb = KB(nc, tc)
        kb.consts = ctx.enter_context(tc.tile_pool(name="consts", bufs=1))
        kb.wpool = ctx.enter_context(tc.tile_pool(name="wpool", bufs=2))
        kb.work = ctx.enter_context(tc.tile_pool(name="work", bufs=1))
        kb.resid = ctx.enter_context(tc.tile_pool(name="resid", bufs=2))
        kb.small = ctx.enter_context(tc.tile_pool(name="small", bufs=2))
        kb.ps_mm = ctx.enter_context(tc.tile_pool(name="ps_mm", bufs=4, space="PSUM"))
        kb.ps_tr = ctx.enter_context(tc.tile_pool(name="ps_tr", bufs=2, space="PSUM"))
        kb.ps_s = ctx.enter_context(tc.tile_pool(name="ps_s", bufs=2, space="PSUM"))

        # inputs needed first: patch conv operands (chunked)
        xp = kb.work.tile([96, T_EFF[0]], DT.bfloat16, tag="pk_qv")
        for qi, (t0c, tnc) in enumerate(_chunks(T_EFF[0])):
            nc.sync.dma_start(out=xp[:, t0c:t0c + tnc],
                              in_=ins['xpatch'][:, t0c:t0c + tnc])
        pw = kb.consts.tile([96, 128], DT.bfloat16, tag="patchw")
        nc.sync.dma_start(out=pw, in_=ins['patchw'][:, :])
        pb = kb.consts.tile([128, 1], DT.float32, tag="patchb")
        nc.sync.dma_start(out=pb, in_=ins['patchb'][:, :])
        kb.ident = kb.consts.tile([128, 128], DT.bfloat16)
        nc.sync.dma_start(out=kb.ident, in_=ins['ident'][:, :])
        kb.ident2 = kb.consts.tile([128, 64], DT.bfloat16)
        nc.sync.dma_start(out=kb.ident2, in_=ins['ident2'][:, :])
        kb.ones = kb.consts.tile([1, 128], DT.bfloat16)
        nc.sync.dma_start(out=kb.ones, in_=ins['ones'][:, :])
        kb._dmaq = 0

        def pdma(out, in_):
            nc.sync.dma_start(out=out, in_=in_)
        
        kb.magic = kb.consts.tile([128, 32], DT.int32)
        nc.sync.dma_start(out=kb.magic, in_=ins['magic'][:, :])

        # prefetch every weight/bias/table/mask/conv at kernel start
        kb.wt, kb.bt, kb.ropes, kb.masks, kb.convs = {}, {}, {}, {}, {}
        for s, D in enumerate(DIMS):
            De, Te = D_EFF[s], T_EFF[s]
            nD = De // 128
            pdim = min(D, 128)
            nT = (TS[s] + 127) // 128
            for bi in range(DEPTHS[s]):
                if s in TS_MODE:
                    w = kb.consts.tile([128, nD, 6, D], DT.bfloat16,
                                       tag=f"w{s}_{bi}")
                    pdma(out=w, in_=ins[f'wts{s}_{bi}'][:, :].rearrange(
                        "p (k l d) -> p k l d", l=6, k=nD))
                    eb = kb.consts.tile([128, nD], DT.float32, tag=f"encb{s}_{bi}")
                    pdma(out=eb, in_=ins[f'encb{s}_{bi}'][:, :])
                    bbc = kb.consts.tile([128, 3, D], DT.bfloat16, tag=f"bbc{s}_{bi}")
                    pdma(out=bbc, in_=ins[f'bbc{s}_{bi}'][:, :]
                                      .rearrange("p (l d) -> p l d", l=3))
                    bsum = kb.consts.tile([128, nT, D], DT.bfloat16,
                                          tag=f"bsum{s}_{bi}")
                    pdma(out=bsum, in_=ins[f'bsum{s}_{bi}'][:, :]
                                      .rearrange("p (j d) -> p j d", j=nT))
                    kb.bt[(s, bi)] = (eb, bbc, bsum)
                    kb.wt[(s, bi)] = w
                    continue
                if True:
                    w = kb.consts.tile(list(ins[f'w{s}_{bi}'].shape), DT.bfloat16,
                                       tag=f"w{s}_{bi}")
                    pdma(out=w, in_=ins[f'w{s}_{bi}'][:, :])
                    b = kb.consts.tile(list(ins[f'bias{s}_{bi}'].shape), DT.float32,
                                       tag=f"bias{s}_{bi}")
                    pdma(out=b, in_=ins[f'bias{s}_{bi}'][:, :])
                    kb.wt[(s, bi)], kb.bt[(s, bi)] = w, b
            if s in TS_MODE:
                ta = kb.consts.tile([128, nT, DIMS[s]], DT.bfloat16, tag=f"ta{s}")
                pdma(out=ta, in_=ins[f'tatm{s}'][:, :, :])
                tb = kb.consts.tile([128, nT, DIMS[s]], DT.bfloat16, tag=f"tb{s}")
                pdma(out=tb, in_=ins[f'tbtm{s}'][:, :, :])
            else:
                ta = kb.consts.tile([128, nD, Te], DT.bfloat16, tag=f"ta{s}")
                pdma(out=ta, in_=ins[f'ta{s}'][:, :, :])
                tb = kb.consts.tile([128, nD, Te], DT.bfloat16, tag=f"tb{s}")
                pdma(out=tb, in_=ins[f'tb{s}'][:, :, :])
            kb.ropes[s] = (ta, tb)
            if s < 3:
                mp = 128 if s == 0 else pdim
                mk = kb.consts.tile([mp, mp], DT.bfloat16, tag=f"mask{s}")
                pdma(out=mk, in_=ins[f'mask{s}'][:, :])
                kb.masks[s] = mk
                Cin, Cout = D, DIMS[s + 1]
                nKi, nMo = max(1, Cin // 128), Cout // 128
                wc = kb.consts.tile([128 if s == 0 else min(Cin, 128),
                                     nKi, 4, nMo, 128],
                                    DT.bfloat16, tag=f"conv{s}")
                pdma(out=wc, in_=ins[f'conv{s}'][:, :].rearrange(
                    "p (a b c d) -> p a b c d", a=nKi, b=4, c=nMo))
                cb = kb.consts.tile([128, nMo], DT.float32, tag=f"convb{s}")
                pdma(out=cb, in_=ins[f'convb{s}'][:, :])
                kb.convs[s] = (wc, cb)

        # ---- patch conv (feature-major, stacked) then to token-major
        T0 = T_EFF[0]
        x1_fm = kb.work.tile([128, 1, T0], DT.bfloat16, tag="t2_fm")
        for t0, tn in _chunks(T0):
            mm = kb.ps_mm.tile([128, NCHUNK], DT.float32, tag="mm_ps")
            nc.tensor.matmul(mm[:128, :tn], lhsT=pw, rhs=xp[:, t0:t0 + tn],
                             start=True, stop=True)
            nc.scalar.activation(out=x1_fm[:, 0, t0:t0 + tn], in_=mm[:128, :tn],
                                 func=AF.Identity, bias=pb, scale=1.0)
        x_tm = _fm_to_tm(kb, x1_fm, T0, 128, tag_dst="x_s0", pool='resid',
                         zero_ragged=True)
        import os
        if os.environ.get('KDBG'):
            nc.gpsimd.dma_start(out=outs['dbg0'][:, :64], in_=x_tm[:, 0, :])
            nc.gpsimd.dma_start(out=outs['dbg1'][:64, :512], in_=x1_fm[:, 0, :512])
            nc.gpsimd.dma_start(out=outs['dbg2'][:48, :512], in_=xp[:, :512])
            kb.dbg_outs = outs

        # ---- stages
        for s, (depth, D) in enumerate(zip(DEPTHS, DIMS)):
            for bi in range(depth):
                if s in TS_MODE:
                    x_tm = _block_ts(kb, ins, s, bi, x_tm)
                else:
                    x_tm = _block(kb, ins, s, bi, x_tm)
            for j, psz in enumerate(_tiles(T_EFF[s])):
                nc.sync.dma_start(out=outs[s][j * 128:j * 128 + psz, :],
                                  in_=x_tm[:psz, j, :])
            if s < 3:
                x_tm = _downsample(kb, ins, s, x_tm)


def _fm_to_tm(kb, fm, T, D, tag_dst, pool='work', add_to=None,
              zero_ragged=False):
    """Feature-major [pdim, nD, T] -> token-major [128, nT, D] via packed PE
    transposes. If add_to is given, the grouped evac is a TT add with it
    (residual fusion) routed DVE; else a grouped copy (DVE/ACT)."""
    nc = kb.nc
    pdim = min(D, 128)
    nD = max(1, D // 128)
    sizes = _tiles(T)
    nT = len(sizes)
    nfull = sum(1 for p in sizes if p == 128)
    dst = getattr(kb, pool).tile([128, nT, D], DT.bfloat16, tag=tag_dst)
    npack = max(1, 1024 // pdim)

    for g in range(nD):
        gsl = slice(g * 128, g * 128 + pdim)
        for j0 in range(0, nfull, npack):
            jn = min(npack, nfull - j0)
            pt = kb.ps_tr.tile([128, 1024], DT.bfloat16, tag="tr_ps")
            for k in range(jn):
                nc.tensor.transpose(
                    pt[:, k * pdim:(k + 1) * pdim],
                    fm[:, g, (j0 + k) * 128:(j0 + k + 1) * 128],
                    kb.ident[:pdim, :pdim])
            src = pt[:, :jn * pdim].rearrange("p (j d) -> p j d", j=jn)
            dsl = dst[:, j0:j0 + jn, gsl]
            if add_to is not None:
                nc.vector.tensor_tensor(out=dsl, in0=src,
                                        in1=add_to[:, j0:j0 + jn, gsl], op=OP.add)
            else:
                kb.evac_copy(dsl, src)
        if nfull < nT:  # one ragged tail tile
            j = nT - 1
            psz = sizes[j]
            if zero_ragged:
                nc.gpsimd.memset(dst[:, j, gsl], 0.0)
            pt = kb.ps_tr.tile([128, 1024], DT.bfloat16, tag="tr_ps")
            nc.tensor.transpose(pt[:psz, :pdim], fm[:, g, j * 128:j * 128 + psz],
                                kb.ident[:pdim, :pdim])
            dsl = dst[:psz, j, gsl]
            if add_to is not None:
                nc.vector.tensor_tensor(out=dsl, in0=pt[:psz, :pdim],
                                        in1=add_to[:psz, j, gsl], op=OP.add)
            else:
                kb.evac_copy(dsl, pt[:psz, :pdim])
    return dst


def _tm_to_fm(kb, tm, T, D, tag_dst, evac_fn=None):
    """Token-major [128, nT, D] -> feature-major [pdim, nD, T]."""
    nc = kb.nc
    pdim = min(D, 128)
    nD = max(1, D // 128)
    sizes = _tiles(T)
    nT = len(sizes)
    fm = kb.work.tile([pdim, nD, T], DT.bfloat16, tag=tag_dst)
    npack = 8  # 8 * 128 cols = 1024 bf16 = one bank
    for g in range(nD):
        for j0 in range(0, nT, npack):
            jn = min(npack, nT - j0)
            pt = kb.ps_tr.tile([128, 1024], DT.bfloat16, tag="tr_ps")
            cols = 0
            for k in range(jn):
                j = j0 + k
                psz = sizes[j]
                nc.tensor.transpose(
                    pt[:pdim, cols:cols + psz],
                    tm[:psz, j, g * 128:g * 128 + pdim],
                    kb.ident[:psz, :psz])
                cols += psz
            if evac_fn is not None:
                evac_fn(fm[:, g, j0 * 128:j0 * 128 + cols], pt[:pdim, :cols], g)
            else:
                kb.evac_copy(fm[:, g, j0 * 128:j0 * 128 + cols], pt[:pdim, :cols])
    return fm


def _ln(kb, x_tm, T, D, halves=1):
    """LayerNorm stats + normalize -> xn_tm bf16. halves=2: stage-1 stacked
    layout, each 128-col row holds two tokens (64 cols each)."""
    nc = kb.nc
    sizes = _tiles(T)
    nTt = len(sizes)
    nT = nTt * halves
    Dh = D // halves
    st = kb.small.tile([128, nT, 6], DT.float32, tag="bnst")
    for j in range(nTt):
        for h in range(halves):
            nc.vector.bn_stats(out=st[:, j * halves + h, :],
                               in_=x_tm[:, j, h * Dh:(h + 1) * Dh])
    mv = kb.small.tile([128, nT, 2], DT.float32, tag="mv")
    for j in range(nT):
        nc.vector.bn_aggr(out=mv[:, j, :], in_=st[:, j, :])
    veps = kb.small.tile([128, nT], DT.float32, tag="veps")
    nc.vector.tensor_scalar(out=veps, in0=mv[:, :, 1], scalar1=EPS,
                            scalar2=None, op0=OP.add)
    h = kb.small.tile([128, nT], DT.int32, tag="hshift")
    nc.vector.tensor_scalar(out=h, in0=veps.bitcast(DT.int32), scalar1=1,
                            scalar2=None, op0=OP.logical_shift_right)
    y = kb.small.tile([128, nT], DT.float32, tag="ynewt")
    nc.vector.tensor_tensor(out=y.bitcast(DT.int32), in0=kb.magic[:, :nT],
                            in1=h, op=OP.subtract)
    r = kb.small.tile([128, nT], DT.float32, tag="rnewt")
    for _ in range(NEWTON_ITERS):
        nc.vector.tensor_tensor(out=r, in0=y, in1=y, op=OP.mult)
        nc.vector.tensor_tensor(out=r, in0=r, in1=veps, op=OP.mult)
        nc.vector.tensor_scalar(out=r, in0=r, scalar1=-0.5, scalar2=1.5,
                                op0=OP.mult, op1=OP.add)
        nc.vector.tensor_tensor(out=y, in0=y, in1=r, op=OP.mult)
    xn_tm = kb.work.tile([128, nTt, D], DT.bfloat16, tag="xn_tm")
    for j, psz in enumerate(sizes):
        for h in range(halves):
            jh = j * halves + h
            nc.vector.tensor_scalar(out=xn_tm[:psz, j, h * Dh:(h + 1) * Dh],
                                    in0=x_tm[:psz, j, h * Dh:(h + 1) * Dh],
                                    scalar1=mv[:psz, jh, 0:1],
                                    scalar2=y[:psz, jh:jh + 1],
                                    op0=OP.subtract, op1=OP.mult)
    return xn_tm


def _block(kb, ins, s, bi, x_tm):
    nc = kb.nc
    D, T = D_EFF[s], T_EFF[s]
    pdim = 128
    nD = max(1, D // 128)
    sizes = _tiles(T)
    nT = len(sizes)
    n = nD

    wt, bt = kb.wt[(s, bi)], kb.bt[(s, bi)]
    ta, tb = kb.ropes[s]
    if s < 3:
        kb.mask = kb.masks[s]

    def wsl(li, ki, mo):
        if D == 64:
            off = {0: 0, 2: 128, 4: 256, 5: 320}
            wid = {0: 128, 2: 128, 4: 64, 5: 64}
            return wt[:, off[li]:off[li] + wid[li]]
        base = li * n * n * 128
        return wt[:, base + (ki * n + mo) * 128: base + (ki * n + mo + 1) * 128]

    def bsl(li, mo, p0=0, p1=128):
        col = {0: 0, 2: 1, 3: 4, 4: 2, 5: 3}[li] if D == 64 else mo * 6 + li
        return bt[p0:p1, col:col + 1]

    def linear(li, in_fm, dst_fn):
        """dst_fn(mm_psum, mo, t0, tn) consumes each output chunk."""
        mrows = {0: 128, 2: 128, 4: 64, 5: 64}[li] if D == 64 else pdim
        for mo in range(n):
            for t0, tn in _chunks(T):
                mm = kb.ps_mm.tile([128, NCHUNK], DT.float32, tag="mm_ps")
                for ki in range(n):
                    nc.tensor.matmul(mm[:mrows, :tn], lhsT=wsl(li, ki, mo),
                                     rhs=in_fm[:, ki, t0:t0 + tn],
                                     start=(ki == 0), stop=(ki == n - 1))
                dst_fn(mm, mo, t0, tn)

    # ---- LN, transpose
    xn_tm = _ln(kb, x_tm, T, D, halves=HALVES[s])
    xn_fm = _tm_to_fm(kb, xn_tm, T, D, tag_dst="xn_fm")

    # ---- enc(relu) / gate(sigmoid)  [packed for D=64]
    xl_fm = kb.work.tile([pdim, nD, T], DT.bfloat16, tag="xl_fm")
    gsig_fm = kb.work.tile([pdim, nD, T], DT.bfloat16, tag="gsig_fm")
    if D == 64:
        def encgate_dst(mm, mo, t0, tn):
            nc.scalar.activation(out=xl_fm[:, 0, t0:t0 + tn], in_=mm[0:64, :tn],
                                 func=AF.Relu, bias=bsl(0, 0, 0, 64), scale=1.0)
            nc.scalar.activation(out=gsig_fm[:, 0, t0:t0 + tn], in_=mm[64:128, :tn],
                                 func=AF.Sigmoid, bias=bsl(0, 0, 64, 128), scale=1.0)
        linear(0, xn_fm, encgate_dst)
    else:
        def enc_dst(mm, mo, t0, tn):
            nc.scalar.activation(out=xl_fm[:, mo, t0:t0 + tn], in_=mm[:pdim, :tn],
                                 func=AF.Relu, bias=bsl(0, mo), scale=1.0)
        linear(0, xn_fm, enc_dst)

        def gate_dst(mm, mo, t0, tn):
            nc.scalar.activation(out=gsig_fm[:, mo, t0:t0 + tn], in_=mm[:pdim, :tn],
                                 func=AF.Sigmoid, bias=bsl(1, mo), scale=1.0)
        linear(1, xn_fm, gate_dst)

    # ---- qk/qks with fused bias+rope-mult; v with bias
    t1_fm = kb.work.tile([pdim, nD, T], DT.bfloat16, tag="t1_fm")
    t2_fm = kb.work.tile([pdim, nD, T], DT.bfloat16, tag="t2_fm")
    if D == 64:
        def qkqks_dst(mm, mo, t0, tn):
            nc.vector.scalar_tensor_tensor(
                out=t1_fm[:, 0, t0:t0 + tn], in0=mm[0:64, :tn],
                scalar=bsl(2, 0, 0, 64), in1=ta[:, 0, t0:t0 + tn],
                op0=OP.add, op1=OP.mult)
            nc.vector.scalar_tensor_tensor(
                out=t2_fm[:, 0, t0:t0 + tn], in0=mm[64:128, :tn],
                scalar=bsl(3, 0, 0, 64), in1=tb[:, 0, t0:t0 + tn],
                op0=OP.add, op1=OP.mult)
        linear(2, xl_fm, qkqks_dst)
    else:
        def qk_dst(mm, mo, t0, tn):
            nc.vector.scalar_tensor_tensor(
                out=t1_fm[:, mo, t0:t0 + tn], in0=mm[:pdim, :tn],
                scalar=bsl(2, mo), in1=ta[:, mo, t0:t0 + tn],
                op0=OP.add, op1=OP.mult)
        linear(2, xl_fm, qk_dst)

        def qks_dst(mm, mo, t0, tn):
            nc.vector.scalar_tensor_tensor(
                out=t2_fm[:, mo, t0:t0 + tn], in0=mm[:pdim, :tn],
                scalar=bsl(3, mo), in1=tb[:, mo, t0:t0 + tn],
                op0=OP.add, op1=OP.mult)
        linear(3, xl_fm, qks_dst)

    if D == 64:
        # pack qkr (rows 0:64) and v (rows 64:128) into one tile: one
        # transpose covers both for the S stage.
        pk = kb.work.tile([128, 1, T], DT.bfloat16, tag="pk_qv")

        def v_dst(mm, mo, t0, tn):
            nc.scalar.activation(out=pk[64:128, 0, t0:t0 + tn], in_=mm[0:64, :tn],
                                 func=AF.Identity, bias=bsl(4, 0, 0, 64), scale=1.0)
        linear(4, xl_fm, v_dst)
        nc.vector.tensor_tensor(out=pk[0:64, 0, :], in0=t1_fm[:, 0, :],
                                in1=t2_fm[:, 0, :], op=OP.add)
        qv_tm = _fm_to_tm(kb, pk, T, 128, tag_dst="xn_tm")
        qkr_fm = pk
    else:
        v_fm = kb.work.tile([pdim, nD, T], DT.bfloat16, tag="v_fm")

        def v_dst(mm, mo, t0, tn):
            nc.scalar.activation(out=v_fm[:, mo, t0:t0 + tn], in_=mm[:pdim, :tn],
                                 func=AF.Identity, bias=bsl(4, mo, 0, pdim), scale=1.0)
        linear(4, xl_fm, v_dst)

        qkr_fm = kb.work.tile([pdim, nD, T], DT.bfloat16, tag="qkr_fm")
        for g in range(nD):
            eng = nc.vector if g % 2 == 0 else nc.gpsimd
            eng.tensor_tensor(out=qkr_fm[:, g, :], in0=t1_fm[:, g, :],
                              in1=t2_fm[:, g, :], op=OP.add)
        qkr_tm = _fm_to_tm(kb, qkr_fm, T, D, tag_dst="qkr_tm")
        v_tm = _fm_to_tm(kb, v_fm, T, D, tag_dst="v_tm")

    # ---- S per feature group + attn
    attn_fm = kb.work.tile([pdim, nD, T], DT.bfloat16, tag="t1_fm")
    for g in range(nD):
        sp = kb.ps_s.tile([128, 128], DT.float32, tag="s_ps")
        for j, psz in enumerate(sizes):
            if D == 64:
                lhsT = qv_tm[:psz, j, 0:64]
                rhs = qv_tm[:psz, j, 64:128]
            else:
                lhsT = qkr_tm[:psz, j, g * 128:g * 128 + pdim]
                rhs = v_tm[:psz, j, g * 128:g * 128 + pdim]
            nc.tensor.matmul(sp[:pdim, :pdim], lhsT=lhsT, rhs=rhs,
                             start=(j == 0), stop=(j == nT - 1))
        sbd = kb.small.tile([128, 128], DT.bfloat16, tag="sbd")
        if s == 0:
            # stacked: S_full = sum of the two diagonal 64-blocks of sp
            ssb = kb.small.tile([128, 128], DT.bfloat16, tag="ssb")
            nc.vector.tensor_tensor(out=ssb, in0=sp, in1=kb.mask, op=OP.mult)
            rp = kb.ps_s.tile([128, 128], DT.float32, tag="s_ps")
            nc.tensor.matmul(rp[:64, :128], lhsT=kb.ident2, rhs=ssb,
                             start=True, stop=True)
            rs = kb.small.tile([64, 128], DT.bfloat16, tag="rssb")
            kb.evac_copy(rs, rp[:64, :128])
            sfull = kb.small.tile([64, 64], DT.bfloat16, tag="sfull")
            nc.vector.tensor_tensor(out=sfull, in0=rs[:, 0:64], in1=rs[:, 64:128],
                                    op=OP.add)
            nc.vector.memset(sbd, 0.0)
            nc.vector.tensor_copy(out=sbd[0:64, 0:64], in_=sfull)
            nc.vector.tensor_copy(out=sbd[64:128, 64:128], in_=sfull)
        elif s < 3:
            nc.vector.tensor_tensor(out=sbd[:pdim, :pdim], in0=sp[:pdim, :pdim],
                                    in1=kb.mask, op=OP.mult)
        else:
            nc.vector.tensor_copy(out=sbd[:pdim, :pdim], in_=sp[:pdim, :pdim])
        for t0, tn in _chunks(T):
            ap = kb.ps_mm.tile([128, NCHUNK], DT.float32, tag="mm_ps")
            nc.tensor.matmul(ap[:pdim, :tn], lhsT=sbd[:pdim, :pdim],
                             rhs=qkr_fm[0:pdim, g, t0:t0 + tn], start=True, stop=True)
            kb.evac_copy(attn_fm[:, g, t0:t0 + tn], ap[:pdim, :tn])

    # ---- out proj: u = (psum + b) * gsig
    u_fm = kb.work.tile([pdim, nD, T], DT.bfloat16, tag="t2_fm")

    def out_dst(mm, mo, t0, tn):
        nc.vector.scalar_tensor_tensor(
            out=u_fm[:, mo, t0:t0 + tn], in0=mm[:pdim, :tn],
            scalar=bsl(5, mo, 0, pdim), in1=gsig_fm[:, mo, t0:t0 + tn],
            op0=OP.add, op1=OP.mult)
    linear(5, attn_fm, out_dst)

    # ---- residual: x_new = x + u^T (fused into transpose evac)
    x_new = _fm_to_tm(kb, u_fm, T, D, tag_dst=f"x_s{s}", pool='resid',
                      add_to=x_tm, zero_ragged=True)
    import os
    if os.environ.get('KDBG') and s == 0 and bi == 0:
        o = kb.dbg_outs
        nc.gpsimd.dma_start(out=o['dbg3'][:, :64], in_=xn_tm[:, 0, :])
        nc.gpsimd.dma_start(out=o['dbg3'][:, 64:128], in_=xn_tm[:, 1, :])
        nc.gpsimd.dma_start(out=o['dbg3'][:, 128:192], in_=x_tm[:, 1, :])
        nc.gpsimd.dma_start(out=o['dbg3'][:, 192:256], in_=x_tm[:, 2, :])
        nc.gpsimd.dma_start(out=o['dbg4'][:64, :512], in_=xl_fm[:, 0, :512])
        nc.gpsimd.dma_start(out=o['dbg4'][64:128, :512], in_=xn_fm[:, 0, :512])
        nc.gpsimd.dma_start(out=o['dbg5'][:64, :512], in_=qkr_fm[:, 0, :512])
        nc.gpsimd.dma_start(out=o['dbg6'][:64, :512], in_=attn_fm[:, 0, :512])
        nc.gpsimd.dma_start(out=o['dbg7'][:64, :512], in_=u_fm[:, 0, :512])
    return x_new




def _block_ts(kb, ins, s, bi, x_tm):
    """Token-stationary block for small-T stages: activations as lhsT,
    weights as moving rhs, most tensors token-major."""
    nc = kb.nc
    D, T = DIMS[s], TS[s]
    nK = D // 128
    sizes = _tiles(T)
    nT = len(sizes)
    wt = kb.wt[(s, bi)]                  # [128, 6, nK, D]
    encb, bbc, bsum = kb.bt[(s, bi)]     # [128,nK] f32, [128,3,D], [128,nT,D]
    ta, tb = kb.ropes[s]                 # token-major [128, nT, D]

    xn_tm = _ln(kb, x_tm, T, D)
    xn_fm = _tm_to_fm(kb, xn_tm, T, D, tag_dst="xn_fm")

    def linear_grp(grp, in_fm, dst_fn):
        gw = len(grp) * D
        for j, psz in enumerate(sizes):
            mm = kb.ps_mm.tile([128, NCHUNK], DT.float32, tag="mm_ps")
            for ki in range(nK):
                nc.tensor.matmul(mm[:psz, :gw],
                                 lhsT=in_fm[:, ki, j * 128:j * 128 + psz],
                                 rhs=wt[:, ki, grp[0]:grp[0] + len(grp), :],
                                 start=(ki == 0), stop=(ki == nK - 1))
            dst_fn(mm, j, psz)

    pair = (D <= 256)

    # enc(+gate): psum -> xlp_tm (copy) -> transpose with fused Relu+bias evac
    xlp_tm = kb.work.tile([128, nT, D], DT.bfloat16, tag="xl_tm")
    gsig_tm = kb.work.tile([128, nT, D], DT.bfloat16, tag="gsig_tm")

    def gate_cols(mm, j, psz, c0):
        nc.vector.tensor_tensor(out=gsig_tm[:psz, j, :], in0=mm[:psz, c0:c0 + D],
                                in1=bbc[:psz, 0, :], op=OP.add)
        nc.scalar.activation(out=gsig_tm[:psz, j, :], in_=gsig_tm[:psz, j, :],
                             func=AF.Sigmoid)

    if pair:
        def encgate_dst(mm, j, psz):
            kb.evac_copy(xlp_tm[:psz, j, :], mm[:psz, 0:D])
            gate_cols(mm, j, psz, D)
        linear_grp([0, 1], xn_fm, encgate_dst)
    else:
        linear_grp([0], xn_fm, lambda mm, j, psz: kb.evac_copy(
            xlp_tm[:psz, j, :], mm[:psz, :D]))
        linear_grp([1], xn_fm, lambda mm, j, psz: gate_cols(mm, j, psz, 0))
    xl_fm = _tm_to_fm(kb, xlp_tm, T, D, tag_dst="xl_fm",
                      evac_fn=lambda dsl, srcp, g: nc.scalar.activation(
                          out=dsl, in_=srcp, func=AF.Relu,
                          bias=encb[:, g:g + 1], scale=1.0))

    # qk/qks: psum * table -> t1/t2 ; v: psum + bias_bc
    t1_tm = kb.work.tile([128, nT, D], DT.bfloat16, tag="t1_tm")
    t2_tm = kb.work.tile([128, nT, D], DT.bfloat16, tag="t2_tm")
    if pair:
        def qkqks_dst(mm, j, psz):
            nc.vector.tensor_tensor(out=t1_tm[:psz, j, :], in0=mm[:psz, 0:D],
                                    in1=ta[:psz, j, :], op=OP.mult)
            nc.vector.tensor_tensor(out=t2_tm[:psz, j, :], in0=mm[:psz, D:2 * D],
                                    in1=tb[:psz, j, :], op=OP.mult)
        linear_grp([2, 3], xl_fm, qkqks_dst)
    else:
        linear_grp([2], xl_fm, lambda mm, j, psz: nc.vector.tensor_tensor(
            out=t1_tm[:psz, j, :], in0=mm[:psz, :D], in1=ta[:psz, j, :],
            op=OP.mult))
        linear_grp([3], xl_fm, lambda mm, j, psz: nc.vector.tensor_tensor(
            out=t2_tm[:psz, j, :], in0=mm[:psz, :D], in1=tb[:psz, j, :],
            op=OP.mult))
    v_tm = kb.work.tile([128, nT, D], DT.bfloat16, tag="v_tm")
    linear_grp([4], xl_fm, lambda mm, j, psz: nc.vector.tensor_tensor(
        out=v_tm[:psz, j, :], in0=mm[:psz, :D], in1=bbc[:psz, 1, :],
        op=OP.add))
    # qkr = t1 + t2 + bsum
    qkr_tm = kb.work.tile([128, nT, D], DT.bfloat16, tag="qkr_tm")
    for j, psz in enumerate(sizes):
        nc.vector.tensor_tensor(out=qkr_tm[:psz, j, :], in0=t1_tm[:psz, j, :],
                                in1=t2_tm[:psz, j, :], op=OP.add)
        nc.vector.tensor_tensor(out=qkr_tm[:psz, j, :], in0=qkr_tm[:psz, j, :],
                                in1=bsum[:psz, j, :], op=OP.add)

    qkr_fm = _tm_to_fm(kb, qkr_tm, T, D, tag_dst="xl_fm")

    # S + attn (fm result)
    attn_fm = kb.work.tile([128, nK, T], DT.bfloat16, tag="t1_tm")
    for g in range(nK):
        sp = kb.ps_s.tile([128, 128], DT.float32, tag="s_ps")
        for j, psz in enumerate(sizes):
            nc.tensor.matmul(sp, lhsT=qkr_tm[:psz, j, g * 128:g * 128 + 128],
                             rhs=v_tm[:psz, j, g * 128:g * 128 + 128],
                             start=(j == 0), stop=(j == nT - 1))
        sbd = kb.small.tile([128, 128], DT.bfloat16, tag="sbd")
        if s < 3:
            nc.vector.tensor_tensor(out=sbd, in0=sp, in1=kb.masks[s], op=OP.mult)
        else:
            nc.vector.tensor_copy(out=sbd, in_=sp)
        ap = kb.ps_mm.tile([128, NCHUNK], DT.float32, tag="mm_ps")
        nc.tensor.matmul(ap[:128, :T], lhsT=sbd, rhs=qkr_fm[:, g, :],
                         start=True, stop=True)
        kb.evac_copy(attn_fm[:, g, :], ap[:128, :T])

    # out proj + bias + gate + residual
    x_new = kb.resid.tile([128, nT, D], DT.bfloat16, tag=f"x_s{s}")
    u_tm = kb.work.tile([128, nT, D], DT.bfloat16, tag="u_tm")

    def out_dst(mm, j, psz):
        if psz < 128:
            nc.gpsimd.memset(x_new[:, j, :], 0.0)
        nc.vector.tensor_tensor(out=u_tm[:psz, j, :], in0=mm[:psz, :D],
                                in1=bbc[:psz, 2, :], op=OP.add)
        nc.vector.tensor_tensor(out=u_tm[:psz, j, :], in0=u_tm[:psz, j, :],
                                in1=gsig_tm[:psz, j, :], op=OP.mult)
        nc.vector.tensor_tensor(out=x_new[:psz, j, :], in0=u_tm[:psz, j, :],
                                in1=x_tm[:psz, j, :], op=OP.add)
    linear_grp([5], attn_fm, out_dst)
    return x_new


def _downsample(kb, ins, s, x_tm):
    nc = kb.nc
    Cin, Cout = DIMS[s], DIMS[s + 1]
    T, Tn = TS[s], TS[s + 1]
    H, Ho = HWS[s], HWS[s + 1]
    pi = min(Cin, 128)
    nKi, nMo = max(1, Cin // 128), Cout // 128

    wc, cb = kb.convs[s]

    if s == 0:
        x_fm = _tm_to_fm(kb, x_tm, T_EFF[0], 128, tag_dst="gsig_fm")
        y_fm = kb.work.tile([128, nMo, Tn], DT.bfloat16, tag="t2_fm")
        for mo in range(nMo):
            for h in range(2):
                mm = kb.ps_mm.tile([128, NCHUNK], DT.float32, tag="mm_ps")
                imv = x_fm[h * 64:(h + 1) * 64, 0, :].rearrange(
                    "p (ho a wo b) -> p ho a wo b", a=2, b=2, ho=14)
                for k, (dy, dx) in enumerate([(0, 0), (0, 1), (1, 0), (1, 1)]):
                    nc.tensor.matmul(mm[:128, :392],
                                     lhsT=wc[h * 64:(h + 1) * 64, 0, k, mo, :],
                                     rhs=imv[:, :, dy, :, dx],
                                     start=(k == 0), stop=(k == 3))
                nc.scalar.activation(out=y_fm[:, mo, h * 392:(h + 1) * 392],
                                     in_=mm[:128, :392], func=AF.Identity,
                                     bias=cb[:, mo:mo + 1], scale=1.0)
        return _fm_to_tm(kb, y_fm, Tn, Cout, tag_dst="x_s1", pool='resid',
                         zero_ragged=True)

    x_fm = _tm_to_fm(kb, x_tm, T, Cin, tag_dst="gsig_fm")
    y_fm = kb.work.tile([128, nMo, Tn], DT.bfloat16, tag="t2_fm")

    rows = max(1, NCHUNK // Ho)
    for mo in range(nMo):
        for y0 in range(0, Ho, rows):
            yn = min(rows, Ho - y0)
            mm = kb.ps_mm.tile([128, NCHUNK], DT.float32, tag="mm_ps")
            first = True
            for ki in range(nKi):
                imv = x_fm[:, ki, :].rearrange("p (ho a wo b) -> p ho a wo b",
                                               a=2, b=2, ho=H // 2)
                for k, (dy, dx) in enumerate([(0, 0), (0, 1), (1, 0), (1, 1)]):
                    nc.tensor.matmul(mm[:128, :yn * Ho], lhsT=wc[:, ki, k, mo, :],
                                     rhs=imv[:, y0:y0 + yn, dy, :, dx],
                                     start=first, stop=(ki == nKi - 1 and k == 3))
                    first = False
            nc.scalar.activation(out=y_fm[:, mo, y0 * Ho:(y0 + yn) * Ho],
                                 in_=mm[:128, :yn * Ho], func=AF.Identity,
                                 bias=cb[:, mo:mo + 1], scale=1.0)

    return _fm_to_tm(kb, y_fm, Tn, Cout, tag_dst=f"x_s{s + 1}", pool='resid',
                     zero_ragged=True)


# ========================================================== public entry

_CACHE = {}


def _get_program():
    if 'nc' not in _CACHE:
        _CACHE['nc'] = build_program()
    return _CACHE['nc']


def make_in_maps(x, params):
    x = np.asarray(x, dtype=np.float32)
    consts = prep_consts(params)
    in_maps = []
    for core in range(8):
        m = dict(consts)
        m['xpatch'] = im2col_patch(x[core % B]).astype(BF16)
        in_maps.append(m)
    return in_maps


def assemble(results):
    feats = []
    for s, D in enumerate(DIMS):
        H = HWS[s]
        imgs = []
        for b in range(B):
            a = np.asarray(results[b][f'feat{s}']).astype(np.float32)
            if s == 0:
                a = np.concatenate([a[:, :64], a[:, 64:]], axis=0)
            imgs.append(a.T.reshape(D, H, H))
        feats.append(np.stack(imgs))
    return tuple(feats)


def kernel(x, params):
    from concourse.bass_utils import run_bass_kernel_spmd
    nc = _get_program()
    in_maps = make_in_maps(x, params)
    res = run_bass_kernel_spmd(nc, in_maps, list(range(8)))
    return assemble(res.results)
